# revision 1
# baseline (speedup 1.0000x reference)
"""Trainium2 Bass kernel for nn_CSPLayer (GNN message passing), 8 NeuronCores.

Strategy: sort edges by src node; core c owns nodes [c*6250,(c+1)*6250) and all
their outgoing edges (scatter over src is then core-local). Per core the edges
are grouped by 128-node tiles, each padded to a fixed 2304 slots so every core
runs an identical instruction stream (SPMD).

Math absorbed on device:
  h   = LN(x);  h0 = (x-mu)*rsqrt(var+eps)   (gamma/beta folded into weights)
  za  = h0 @ (gamma*Wa)           (own nodes, SBUF resident, bf16)
  zb  = h0 @ (gamma*Wb)           (all nodes, DRAM table, bf16, gathered by dst)
  wlat[g] = (L L^T)[g] @ Wl + be1 + beta@(Wa+Wb)   (gathered by edge2graph)
  z1T[:,e] = za[src] (stair-matmul) + zb[dst]^T + wlat[g]^T + mod1(fj-fi)@Wf
  e1 = silu(z1); e2 = silu(e1@We2+be2); agg = scatter-mean over src
  n  = silu(silu([h|agg]@Wn1+bn1)@Wn2+bn2);  out = x + n
"""

import os
import sys

import numpy as np

if "/opt/trn_rl_repo" not in sys.path:
    sys.path.insert(0, "/opt/trn_rl_repo")

import concourse.bass as bass
import concourse.bacc as bacc
import concourse.mybir as mybir
import concourse.tile as tile
from concourse.masks import make_identity

F32 = mybir.dt.float32
BF16 = mybir.dt.bfloat16
FP16 = mybir.dt.float16
I32 = mybir.dt.int32

N, E, G, H = 50000, 800000, 128, 128
NC = 8
NPC = N // NC            # 6250 nodes per core
NT = 49                  # node tiles per core (48*128 + 106)
ENT = 2304               # padded edge slots per node tile (18 subchunks)
SNT = ENT // 128         # 18 subchunks of 128 edges
# FM chunk plan: (subchunk offset j0, subchunk count S)
CHUNKS = [(0, 4), (4, 4), (8, 4), (12, 4), (16, 2)]
NCHUNK = len(CHUNKS)
EPS = 1e-5
AF = mybir.ActivationFunctionType
OP = mybir.AluOpType


# --------------------------------------------------------------------------
# host-side prep: pure index manipulation / padding / layout
# --------------------------------------------------------------------------

def _prep_core(c, srcS, dstS, e2gS, deg):
    m0 = c * NPC
    lo = np.searchsorted(srcS, m0)
    hi = np.searchsorted(srcS, m0 + NPC)
    s_c, d_c, g_c = srcS[lo:hi], dstS[lo:hi], e2gS[lo:hi]

    nb = np.searchsorted(s_c, m0 + 128 * np.arange(NT + 1))
    dstT = np.zeros((NT, 128, SNT), np.int32)
    e2gT = np.zeros((NT, 128, SNT), np.int32)
    fidxT = np.zeros((NT, 128, 2 * SNT), np.int32)
    srclocT = np.full((NT, 128, SNT), 200.0, np.float32)
    degsrcT = np.ones((NT, 128, SNT), np.float32)
    stairsT = np.zeros((NT, 128, 2 * NCHUNK), np.float32)

    for nt in range(NT):
        a, b = nb[nt], nb[nt + 1]
        cnt = b - a
        assert cnt <= ENT, f"node tile overflow: {cnt} > {ENT}"
        sl = s_c[a:b]

        def pack(vals, fill, dtype):
            buf = np.full(ENT, fill, dtype)
            buf[:cnt] = vals
            return buf.reshape(SNT, 128).T  # [128, SNT]

        dstT[nt] = pack(d_c[a:b], 0, np.int32)
        e2gT[nt] = pack(g_c[a:b], 0, np.int32)
        fidxT[nt, :, :SNT] = pack(sl, 0, np.int32)
        fidxT[nt, :, SNT:] = dstT[nt]
        srclocT[nt] = pack(sl - (m0 + 128 * nt), 200.0, np.float32)
        degsrcT[nt] = pack(np.maximum(deg[sl], 1), 1.0, np.float32)

        node_ids = m0 + 128 * nt + np.arange(128)
        st = np.searchsorted(sl, node_ids)
        en = np.searchsorted(sl, node_ids + 1)
        for ci, (j0, S) in enumerate(CHUNKS):
            base, w = j0 * 128, S * 128
            stairsT[nt, :, 2 * ci] = np.clip(st - base, 0, w)
            stairsT[nt, :, 2 * ci + 1] = np.clip(en - base, 0, w)
    return dict(dstT=dstT, e2gT=e2gT, fidxT=fidxT, srclocT=srclocT,
                degsrcT=degsrcT, stairsT=stairsT)


def _host_prep(inputs):
    src = np.asarray(inputs["edge_index"][0], dtype=np.int64)
    dst = np.asarray(inputs["edge_index"][1], dtype=np.int64)
    e2g = np.asarray(inputs["edge2graph"], dtype=np.int64)
    deg = np.bincount(src, minlength=N)
    perm = np.argsort(src, kind="stable")
    srcS, dstS, e2gS = src[perm], dst[perm], e2g[perm]
    return [_prep_core(c, srcS, dstS, e2gS, deg) for c in range(NC)], deg


# --------------------------------------------------------------------------
# bass program (single SPMD program for all 8 cores)
# --------------------------------------------------------------------------

def build_program():
    nc = bacc.Bacc()
    p = lambda n, s, d: nc.declare_dram_parameter(n, list(s), d, isOutput=False)

    x = p("x", (N, H), F32)
    frac16 = p("frac16", (N, 4), FP16)
    lat9 = p("lat9", (G, 9), F32)
    Wa = p("Wa", (H, H), F32)
    Wb = p("Wb", (H, H), F32)
    Wl = p("Wl", (9, H), F32)
    Wf = p("Wf", (3, H), F32)
    be1 = p("be1", (H,), F32)
    We2 = p("We2", (H, H), F32)
    be2 = p("be2", (H,), F32)
    Wn1h = p("Wn1h", (H, H), F32)
    Wn1a = p("Wn1a", (H, H), F32)
    bn1 = p("bn1", (H,), F32)
    Wn2 = p("Wn2", (H, H), F32)
    bn2 = p("bn2", (H,), F32)
    gamma = p("gamma", (H,), F32)
    beta = p("beta", (H,), F32)
    m0v = p("m0v", (1,), I32)  # unused on device math path; per-core base (doc)
    dstT = p("dstT", (NT, 128, SNT), I32)
    e2gT = p("e2gT", (NT, 128, SNT), I32)
    fidxT = p("fidxT", (NT, 128, 2 * SNT), I32)
    srclocT = p("srclocT", (NT, 128, SNT), F32)
    degsrcT = p("degsrcT", (NT, 128, SNT), F32)
    stairsT = p("stairsT", (NT, 128, 2 * NCHUNK), F32)
    xown = p("xown", (NPC, H), F32)

    out = nc.declare_dram_parameter("out", [NPC, H], F32, isOutput=True)

    zb_tbl = nc.dram_tensor("zb_tbl", [N, H], BF16)
    wlat_tbl = nc.dram_tensor("wlat_tbl", [G, H], BF16)

    NGT = (N + 127) // 128  # 391 global tiles

    with tile.TileContext(nc) as tc:
        with tc.tile_pool(name="persist", bufs=1) as pp:
            # ---------------- constants ----------------
            I_bf = pp.tile([128, 128], BF16)
            make_identity(nc, I_bf[:])
            I_f32 = pp.tile([128, 128], F32)
            make_identity(nc, I_f32[:])
            iota_i = pp.tile([128, 512], I32)
            nc.gpsimd.iota(iota_i[:], pattern=[[1, 512]], base=0,
                           channel_multiplier=0)
            iota_f = pp.tile([128, 512], F32)
            nc.any.tensor_copy(out=iota_f[:], in_=iota_i[:])

            gcol = pp.tile([128, 1], F32)
            nc.sync.dma_start(out=gcol[:], in_=gamma[:, None])
            bcol = pp.tile([128, 1], F32)
            nc.sync.dma_start(out=bcol[:], in_=beta[:, None])

            def load_col(name_ap, tag):
                t = pp.tile([128, 1], F32, tag=tag)
                nc.sync.dma_start(out=t[:], in_=name_ap[:, None])
                return t

            be2c = load_col(be2, "be2c")
            bn2c = load_col(bn2, "bn2c")
            bn1c_raw = load_col(bn1, "bn1craw")
            epsc = pp.tile([128, 1], F32)
            nc.gpsimd.memset(epsc[:], EPS)

            # f32 weights staged in SBUF
            def load_w(ap, shape, tag):
                t = pp.tile(list(shape), F32, tag=tag)
                nc.sync.dma_start(out=t[:], in_=ap[:, :])
                return t

            Wa_f = load_w(Wa, (128, 128), "Wa_f")
            Wb_f = load_w(Wb, (128, 128), "Wb_f")
            Wl_f = load_w(Wl, (9, 128), "Wl_f")
            Wf_f = load_w(Wf, (3, 128), "Wf_f")
            We2_f = load_w(We2, (128, 128), "We2_f")
            Wn1h_f = load_w(Wn1h, (128, 128), "Wn1h_f")
            Wn1a_f = load_w(Wn1a, (128, 128), "Wn1a_f")
            Wn2_f = load_w(Wn2, (128, 128), "Wn2_f")
            lat_f = load_w(lat9, (128, 9), "lat_f")

            # gamma-folded bf16 weights (ACT: out = in * scale, scale per-
            # partition; TS encoding only allows one sync wait so avoid DVE TS)
            Wap_bf = pp.tile([128, 128], BF16)
            nc.scalar.activation(Wap_bf[:], Wa_f[:], AF.Identity,
                                 scale=gcol[:])
            Wbp_bf = pp.tile([128, 128], BF16)
            nc.scalar.activation(Wbp_bf[:], Wb_f[:], AF.Identity,
                                 scale=gcol[:])
            Wn1h_bf = pp.tile([128, 128], BF16)
            nc.scalar.activation(Wn1h_bf[:], Wn1h_f[:], AF.Identity,
                                 scale=gcol[:])
            We2_bf = pp.tile([128, 128], BF16)
            nc.any.tensor_copy(out=We2_bf[:], in_=We2_f[:])
            Wn1a_bf = pp.tile([128, 128], BF16)
            nc.any.tensor_copy(out=Wn1a_bf[:], in_=Wn1a_f[:])
            Wn2_bf = pp.tile([128, 128], BF16)
            nc.any.tensor_copy(out=Wn2_bf[:], in_=Wn2_f[:])
            Wf4_bf = pp.tile([4, 128], BF16)
            nc.gpsimd.memset(Wf4_bf[:], 0.0)
            nc.any.tensor_copy(out=Wf4_bf[:3, :], in_=Wf_f[:])

            ones1 = pp.tile([1, 128], F32)
            nc.gpsimd.memset(ones1[:], 1.0)
            ones18 = pp.tile([128, SNT], F32)
            nc.gpsimd.memset(ones18[:], 1.0)

            # persistent per-core state
            za_own = pp.tile([128, NT, 128], BF16)
            h0T_own = pp.tile([128, NT, 128], BF16)
            x_own = pp.tile([128, NT, 128], F32)
            nc.gpsimd.memset(za_own[:], 0.0)
            nc.gpsimd.memset(h0T_own[:], 0.0)

            with (
                tc.tile_pool(name="pre", bufs=3) as pl,
                tc.tile_pool(name="oncepsum", bufs=1, space="PSUM") as pon,
            ):
                # ---- small one-time: wlat table, bias folds ----
                Wab = pl.tile([128, 128], F32)
                nc.vector.tensor_tensor(out=Wab[:], in0=Wa_f[:], in1=Wb_f[:],
                                        op=OP.add)
                ps_r = pon.tile([1, 128], F32)
                nc.tensor.matmul(ps_r[:], lhsT=bcol[:], rhs=Wab[:],
                                 start=True, stop=True)
                be1row = pl.tile([1, 128], F32)
                nc.sync.dma_start(out=be1row[:], in_=be1[None, :])
                be1tot = pp.tile([1, 128], F32)
                nc.vector.tensor_tensor(out=be1tot[:], in0=ps_r[:],
                                        in1=be1row[:], op=OP.add)

                ps_c = pon.tile([128, 1], F32)
                nc.tensor.matmul(ps_c[:], lhsT=Wn1h_f[:], rhs=bcol[:],
                                 start=True, stop=True)
                bn1c = pp.tile([128, 1], F32)
                nc.vector.tensor_tensor(out=bn1c[:], in0=ps_c[:],
                                        in1=bn1c_raw[:], op=OP.add)

                # lat_ip = L @ L^T per graph -> [G, 9]
                latip = pl.tile([128, 9], F32)
                for i in range(3):
                    for k in range(3):
                        tmp = pl.tile([128, 3], F32, tag="latmp")
                        nc.vector.tensor_tensor(
                            out=tmp[:], in0=lat_f[:, 3 * i:3 * i + 3],
                            in1=lat_f[:, 3 * k:3 * k + 3], op=OP.mult)
                        nc.vector.tensor_reduce(
                            out=latip[:, 3 * i + k:3 * i + k + 1], in_=tmp[:],
                            op=OP.add, axis=mybir.AxisListType.X)
                ps_lt = pon.tile([9, 128], F32)
                nc.tensor.transpose(ps_lt[:], latip[:], I_f32[:])
                latipT = pl.tile([9, 128], F32)
                nc.any.tensor_copy(out=latipT[:], in_=ps_lt[:])
                ps_wl = pon.tile([128, 128], F32)
                nc.tensor.matmul(ps_wl[:], lhsT=latipT[:], rhs=Wl_f[:],
                                 start=True, stop=False)
                nc.tensor.matmul(ps_wl[:], lhsT=ones1[:, :G], rhs=be1tot[:],
                                 start=False, stop=True)
                wlat_bf = pl.tile([128, 128], BF16)
                nc.any.tensor_copy(out=wlat_bf[:], in_=ps_wl[:])
                nc.sync.dma_start(out=wlat_tbl[:, :], in_=wlat_bf[:])

            with (
                tc.tile_pool(name="pre2", bufs=3) as pl,
                tc.tile_pool(name="prepsum", bufs=2, space="PSUM") as pps,
                tc.tile_pool(name="prepsum1", bufs=2, space="PSUM") as pps1,
            ):
                # ---- phase 1a: zb table for all N nodes ----
                for t in range(NGT):
                    r0 = t * 128
                    rows = min(128, N - r0)
                    xt = pl.tile([128, 128], F32, tag="xt")
                    nc.sync.dma_start(out=xt[:rows, :], in_=x[r0:r0 + rows, :])
                    st6 = pl.tile([128, 6], F32, tag="st6")
                    nc.vector.bn_stats(st6[:rows, :], xt[:rows, :])
                    st2 = pl.tile([128, 2], F32, tag="st2")
                    nc.vector.bn_aggr(st2[:rows, :], st6[:rows, :])
                    sd = pl.tile([128, 1], F32, tag="sd")
                    nc.scalar.activation(sd[:rows, :], st2[:rows, 1:2],
                                         AF.Sqrt, bias=epsc[:rows, :])
                    a = pl.tile([128, 1], F32, tag="a")
                    nc.vector.reciprocal(a[:rows, :], sd[:rows, :])
                    bnn = pl.tile([128, 1], F32, tag="bnn")
                    nc.vector.tensor_scalar(bnn[:rows, :], st2[:rows, 0:1],
                                            a[:rows, :], -1.0, OP.mult, OP.mult)
                    h0 = pl.tile([128, 128], BF16, tag="h0")
                    nc.scalar.activation(h0[:rows, :], xt[:rows, :],
                                         AF.Identity, bias=bnn[:rows, :],
                                         scale=a[:rows, :])
                    ps_t = pps.tile([128, 128], BF16, tag="psT")
                    nc.tensor.matmul(ps_t[:, :rows], h0[:rows, :],
                                     I_bf[:rows, :rows],
                                     is_transpose=True, start=True, stop=True)
                    h0T = pl.tile([128, 128], BF16, tag="h0T")
                    nc.any.tensor_copy(out=h0T[:, :rows], in_=ps_t[:, :rows])
                    ps_zb = pps1.tile([128, 128], F32, tag="pszb")
                    nc.tensor.matmul(ps_zb[:rows, :], lhsT=h0T[:, :rows],
                                     rhs=Wbp_bf[:], start=True, stop=True)
                    zb_bf = pl.tile([128, 128], BF16, tag="zbbf")
                    nc.any.tensor_copy(out=zb_bf[:rows, :], in_=ps_zb[:rows, :])
                    nc.sync.dma_start(out=zb_tbl[r0:r0 + rows, :],
                                      in_=zb_bf[:rows, :])

                # ---- phase 1b: own nodes -> za_own, h0T_own, x_own ----
                for nt in range(NT):
                    rows = 106 if nt == NT - 1 else 128
                    xt = pl.tile([128, 128], F32, tag="xt")
                    nc.sync.dma_start(out=xt[:rows, :],
                                      in_=xown[nt * 128:nt * 128 + rows, :])
                    nc.any.tensor_copy(out=x_own[:rows, nt, :],
                                       in_=xt[:rows, :])
                    st6 = pl.tile([128, 6], F32, tag="st6")
                    nc.vector.bn_stats(st6[:rows, :], xt[:rows, :])
                    st2 = pl.tile([128, 2], F32, tag="st2")
                    nc.vector.bn_aggr(st2[:rows, :], st6[:rows, :])
                    sd = pl.tile([128, 1], F32, tag="sd")
                    nc.scalar.activation(sd[:rows, :], st2[:rows, 1:2],
                                         AF.Sqrt, bias=epsc[:rows, :])
                    a = pl.tile([128, 1], F32, tag="a")
                    nc.vector.reciprocal(a[:rows, :], sd[:rows, :])
                    bnn = pl.tile([128, 1], F32, tag="bnn")
                    nc.vector.tensor_scalar(bnn[:rows, :], st2[:rows, 0:1],
                                            a[:rows, :], -1.0, OP.mult, OP.mult)
                    h0 = pl.tile([128, 128], BF16, tag="h0")
                    nc.scalar.activation(h0[:rows, :], xt[:rows, :],
                                         AF.Identity, bias=bnn[:rows, :],
                                         scale=a[:rows, :])
                    ps_t = pps.tile([128, 128], BF16, tag="psT")
                    nc.tensor.matmul(ps_t[:, :rows], h0[:rows, :],
                                     I_bf[:rows, :rows],
                                     is_transpose=True, start=True, stop=True)
                    nc.any.tensor_copy(out=h0T_own[:, nt, :rows],
                                       in_=ps_t[:, :rows])
                    ps_za = pps1.tile([128, 128], F32, tag="psza")
                    nc.tensor.matmul(ps_za[:rows, :],
                                     lhsT=h0T_own[:, nt, :rows],
                                     rhs=Wap_bf[:], start=True, stop=True)
                    nc.any.tensor_copy(out=za_own[:rows, nt, :],
                                       in_=ps_za[:rows, :])

            # ---------------- phase 2: edges + node update ----------------
            with (
                tc.tile_pool(name="idx", bufs=2) as pidx,
                tc.tile_pool(name="gat", bufs=2) as pg,
                tc.tile_pool(name="work", bufs=2) as pw,
                tc.tile_pool(name="ps_z1", bufs=2, space="PSUM") as ps_z1,
                tc.tile_pool(name="ps_z2", bufs=2, space="PSUM") as ps_z2,
                tc.tile_pool(name="ps_agg", bufs=2, space="PSUM") as ps_agg,
                tc.tile_pool(name="ps_sm", bufs=2, space="PSUM") as ps_sm,
            ):
                for nt in range(NT):
                    rows = 106 if nt == NT - 1 else 128
                    # ---- index loads ----
                    t_dst = pidx.tile([128, SNT], I32, tag="dst")
                    nc.sync.dma_start(out=t_dst[:], in_=dstT[nt, :, :])
                    t_e2g = pidx.tile([128, SNT], I32, tag="e2g")
                    nc.sync.dma_start(out=t_e2g[:], in_=e2gT[nt, :, :])
                    t_fid = pidx.tile([128, 2 * SNT], I32, tag="fid")
                    nc.sync.dma_start(out=t_fid[:], in_=fidxT[nt, :, :])
                    t_srl = pidx.tile([128, SNT], F32, tag="srl")
                    nc.sync.dma_start(out=t_srl[:], in_=srclocT[nt, :, :])
                    t_deg = pidx.tile([128, SNT], F32, tag="deg")
                    nc.sync.dma_start(out=t_deg[:], in_=degsrcT[nt, :, :])
                    t_str = pidx.tile([128, 2 * NCHUNK], F32, tag="str")
                    nc.sync.dma_start(out=t_str[:], in_=stairsT[nt, :, :])

                    t_inv = pidx.tile([128, SNT], F32, tag="inv")
                    nc.vector.reciprocal(t_inv[:], t_deg[:])

                    # ---- gathers (edge-major, one row per partition) ----
                    g_zb = pg.tile([128, SNT, 128], BF16, tag="gzb")
                    g_wl = pg.tile([128, SNT, 128], BF16, tag="gwl")
                    g_fr = pg.tile([128, 2 * SNT, 4], FP16, tag="gfr")
                    for j in range(SNT):
                        nc.gpsimd.indirect_dma_start(
                            out=g_zb[:, j, :], out_offset=None, in_=zb_tbl[:, :],
                            in_offset=bass.IndirectOffsetOnAxis(
                                ap=t_dst[:, j:j + 1], axis=0))
                        nc.gpsimd.indirect_dma_start(
                            out=g_wl[:, j, :], out_offset=None,
                            in_=wlat_tbl[:, :],
                            in_offset=bass.IndirectOffsetOnAxis(
                                ap=t_e2g[:, j:j + 1], axis=0))
                        nc.gpsimd.indirect_dma_start(
                            out=g_fr[:, j, :], out_offset=None,
                            in_=frac16[:, :],
                            in_offset=bass.IndirectOffsetOnAxis(
                                ap=t_fid[:, j:j + 1], axis=0))
                        nc.gpsimd.indirect_dma_start(
                            out=g_fr[:, SNT + j, :], out_offset=None,
                            in_=frac16[:, :],
                            in_offset=bass.IndirectOffsetOnAxis(
                                ap=t_fid[:, SNT + j:SNT + j + 1], axis=0))

                    agg = ps_agg.tile([128, 128], F32, tag="agg")

                    for ci, (j0, S) in enumerate(CHUNKS):
                        W = S * 128
                        # staircase selection matrix selT [128n, W]
                        t0 = pw.tile([128, 512], BF16, tag="t0")
                        nc.vector.tensor_scalar(
                            t0[:, :W], iota_f[:, :W],
                            t_str[:, 2 * ci + 1:2 * ci + 2], None, OP.is_lt)
                        selT = pw.tile([128, 512], BF16, tag="selT")
                        nc.vector.scalar_tensor_tensor(
                            out=selT[:, :W], in0=iota_f[:, :W],
                            scalar=t_str[:, 2 * ci:2 * ci + 1],
                            in1=t0[:, :W], op0=OP.is_ge, op1=OP.mult)

                        # zb + wlat summed, then xbar-transposed to FM
                        gsum = pw.tile([128, 4, 128], BF16, tag="gsum")
                        nc.vector.tensor_tensor(
                            out=gsum[:, :S, :], in0=g_zb[:, j0:j0 + S, :],
                            in1=g_wl[:, j0:j0 + S, :], op=OP.add)
                        gT = pw.tile([128, 4, 128], BF16, tag="gT")
                        nc.sync.dma_start_transpose(gT[:, :S, :],
                                                    gsum[:, :S, :])

                        # frac: dmod = python_mod(fj - fi, 1)
                        dmf = pw.tile([128, 16], F32, tag="dmf")
                        nc.vector.tensor_tensor(
                            out=dmf[:, :4 * S],
                            in0=g_fr[:, SNT + j0:SNT + j0 + S, :],
                            in1=g_fr[:, j0:j0 + S, :], op=OP.subtract)
                        # mod1 for x in (-1,1): x + (x < 0)
                        dneg = pw.tile([128, 16], F32, tag="dneg")
                        nc.vector.tensor_scalar(dneg[:, :4 * S], dmf[:, :4 * S],
                                                0.0, None, OP.is_lt)
                        dmb = pw.tile([128, 16], BF16, tag="dmb")
                        nc.vector.tensor_tensor(out=dmb[:, :4 * S],
                                                in0=dmf[:, :4 * S],
                                                in1=dneg[:, :4 * S], op=OP.add)
                        fdT = pw.tile([4, 4, 128], BF16, tag="fdT")
                        for j in range(S):
                            ps_fd = ps_sm.tile([4, 128], BF16, tag="psfd")
                            nc.tensor.matmul(ps_fd[:], dmb[:, 4 * j:4 * j + 4],
                                             I_bf[:], is_transpose=True,
                                             start=True, stop=True)
                            nc.any.tensor_copy(out=fdT[:, j, :], in_=ps_fd[:])

                        # z1T accumulation [128H, W]
                        z1 = ps_z1.tile([128, 512], F32, tag="z1")
                        nc.tensor.matmul(z1[:, :W], lhsT=za_own[:, nt, :],
                                         rhs=selT[:, :W], start=True,
                                         stop=False, skip_group_check=True)
                        nc.tensor.matmul(z1[:, :W], lhsT=I_bf[:],
                                         rhs=gT[:, :S, :], start=False,
                                         stop=False, skip_group_check=True)
                        for j in range(S):
                            nc.tensor.matmul(
                                z1[:, j * 128:(j + 1) * 128], lhsT=Wf4_bf[:],
                                rhs=fdT[:, j, :], start=False,
                                stop=(j == S - 1), skip_group_check=True)

                        e1T = pw.tile([128, 512], BF16, tag="e1T")
                        nc.scalar.activation(e1T[:, :W], z1[:, :W], AF.Silu)

                        z2 = ps_z2.tile([128, 512], F32, tag="z2")
                        nc.tensor.matmul(z2[:, :W], lhsT=We2_bf[:],
                                         rhs=e1T[:, :W], start=True, stop=True)
                        e2T = pw.tile([128, 512], BF16, tag="e2T")
                        nc.scalar.activation(e2T[:, :W], z2[:, :W], AF.Silu,
                                             bias=be2c[:])
                        e2em = pw.tile([128, 4, 128], BF16, tag="e2em")
                        nc.sync.dma_start_transpose(e2em[:, :S, :], e2T[:, :W])

                        # scatter-mean matmuls into agg [128H, 128n]
                        for j in range(S):
                            jj = j0 + j
                            selp = pw.tile([128, 128], BF16, tag="selp")
                            nc.vector.tensor_scalar(
                                selp[:], iota_f[:, :128],
                                t_srl[:, jj:jj + 1], t_inv[:, jj:jj + 1],
                                OP.is_equal, OP.mult)
                            nc.tensor.matmul(
                                agg[:], lhsT=e2em[:, j, :], rhs=selp[:],
                                start=(ci == 0 and j == 0),
                                stop=(ci == NCHUNK - 1 and j == S - 1),
                                skip_group_check=True)

                    # ---- node update for this tile ----
                    aggb = pw.tile([128, 128], BF16, tag="aggb")
                    nc.any.tensor_copy(out=aggb[:], in_=agg[:])
                    n1 = ps_z1.tile([128, 512], F32, tag="z1")
                    nc.tensor.matmul(n1[:, :128], lhsT=Wn1h_bf[:],
                                     rhs=h0T_own[:, nt, :], start=True,
                                     stop=False, skip_group_check=True)
                    nc.tensor.matmul(n1[:, :128], lhsT=Wn1a_bf[:], rhs=aggb[:],
                                     start=False, stop=True,
                                     skip_group_check=True)
                    n1T = pw.tile([128, 128], BF16, tag="n1T")
                    nc.scalar.activation(n1T[:], n1[:, :128], AF.Silu,
                                         bias=bn1c[:])
                    n2 = ps_z2.tile([128, 512], F32, tag="z2")
                    nc.tensor.matmul(n2[:, :128], lhsT=Wn2_bf[:], rhs=n1T[:],
                                     start=True, stop=True)
                    n2T = pw.tile([128, 128], BF16, tag="n2T")
                    nc.scalar.activation(n2T[:], n2[:, :128], AF.Silu,
                                         bias=bn2c[:])
                    n2em = pw.tile([128, 1, 128], BF16, tag="n2em")
                    nc.sync.dma_start_transpose(n2em[:], n2T[:])
                    ot = pw.tile([128, 128], F32, tag="ot")
                    nc.vector.tensor_tensor(out=ot[:rows, :],
                                            in0=x_own[:rows, nt, :],
                                            in1=n2em[:rows, 0, :], op=OP.add)
                    nc.sync.dma_start(out=out[nt * 128:nt * 128 + rows, :],
                                      in_=ot[:rows, :])
    nc.finalize()
    return nc


_PROGRAM = None


def kernel(**inputs) -> np.ndarray:
    out, _ = run(inputs, trace=False)
    return out


def run(inputs, trace=False):
    global _PROGRAM
    from concourse.bass_utils import run_bass_kernel_spmd

    cores, deg = _host_prep(inputs)

    x = np.ascontiguousarray(np.asarray(inputs["node_features"], np.float32))
    frac = np.asarray(inputs["frac_coords"], np.float32)
    frac16 = np.zeros((N, 4), np.float16)
    frac16[:, :3] = frac.astype(np.float16)
    lat = np.asarray(inputs["lattices"], np.float32).reshape(G, 9)
    We1 = np.asarray(inputs["We1"], np.float32)
    Wn1 = np.asarray(inputs["Wn1"], np.float32)

    common = dict(
        x=x, frac16=frac16, lat9=np.ascontiguousarray(lat),
        Wa=np.ascontiguousarray(We1[0:128]),
        Wb=np.ascontiguousarray(We1[128:256]),
        Wl=np.ascontiguousarray(We1[256:265]),
        Wf=np.ascontiguousarray(We1[265:268]),
        be1=np.asarray(inputs["be1"], np.float32),
        We2=np.asarray(inputs["We2"], np.float32),
        be2=np.asarray(inputs["be2"], np.float32),
        Wn1h=np.ascontiguousarray(Wn1[0:128]),
        Wn1a=np.ascontiguousarray(Wn1[128:256]),
        bn1=np.asarray(inputs["bn1"], np.float32),
        Wn2=np.asarray(inputs["Wn2"], np.float32),
        bn2=np.asarray(inputs["bn2"], np.float32),
        gamma=np.asarray(inputs["gamma"], np.float32),
        beta=np.asarray(inputs["beta"], np.float32),
    )

    in_maps = []
    for c in range(NC):
        d = dict(common)
        d.update(
            m0v=np.array([c * NPC], np.int32),
            dstT=cores[c]["dstT"], e2gT=cores[c]["e2gT"],
            fidxT=cores[c]["fidxT"], srclocT=cores[c]["srclocT"],
            degsrcT=cores[c]["degsrcT"], stairsT=cores[c]["stairsT"],
            xown=np.ascontiguousarray(x[c * NPC:(c + 1) * NPC]),
        )
        in_maps.append(d)

    if _PROGRAM is None:
        _PROGRAM = build_program()

    res = run_bass_kernel_spmd(_PROGRAM, in_maps, list(range(NC)), trace=trace)
    outp = np.concatenate([res.results[c]["out"] for c in range(NC)], axis=0)
    return outp.astype(np.float32), res


if __name__ == "__main__":
    build_program()
    print("program built OK")



# revision 3
# speedup vs baseline: 6.5214x; 6.5214x over previous
"""Trainium2 Bass kernel for nn_CSPLayer (GNN message passing), 8 NeuronCores.

Strategy: sort edges by src node; core c owns nodes [c*6250,(c+1)*6250) and all
their outgoing edges (scatter over src is then core-local). Per core the edges
are grouped by 128-node tiles, each padded to a fixed 2304 slots so every core
runs an identical instruction stream (SPMD).

v2 pipeline changes vs v1:
  - x is sharded (each core receives only its own 6250 rows); every core
    computes the zb table for its own nodes and an on-device AllGather
    builds the full [N,H] zb table each core gathers from.  This removes
    the 8x-replicated 25.6MB x input (205MB -> 25.6MB host->device).
  - frac terms are folded into per-node tables: frac_diff = fj - fi + k
    with k in {0,1}^3 the mod-1 wrap bits (computed exactly on host).
    fj@Wf folds into zb, -fi@Wf into za, and k@Wf + lat_ip@Wl + be1tot
    into a 1024-row combined table indexed by (k*128 + graph).  This
    eliminates the frac gathers and all per-edge frac math.
  - gamma/beta/biases folded into bf16 weights on the host.
  - jit runner is cached module-wide (trace/lower/compile once); donated
    output zero-buffers are created on device; output is n (bf16), the
    residual x + n is added on the host in f32.

Math:
  h   = LN(x);  h0 = (x-mu)*rsqrt(var+eps)   (gamma/beta folded into weights)
  za  = h0 @ (gamma*Wa) - frac @ Wf          (own nodes, SBUF resident, bf16)
  zb  = h0 @ (gamma*Wb) + frac @ Wf          (own slice -> AllGather -> [N,H])
  comb[k*128+g] = (L L^T)[g] @ Wl + be1 + beta@(Wa+Wb) + k @ Wf
  z1T[:,e] = za[src] (stair-matmul) + zb[dst]^T + comb[kcode,e2g]^T
  e1 = silu(z1); e2 = silu(e1@We2+be2); agg = scatter-mean over src
  n  = silu(silu([h|agg]@Wn1+bn1)@Wn2+bn2);  out = x + n (host add)
"""

import os
import sys

import numpy as np

if "/opt/trn_rl_repo" not in sys.path:
    sys.path.insert(0, "/opt/trn_rl_repo")

import concourse.bass as bass
import concourse.bacc as bacc
import concourse.mybir as mybir
import concourse.tile as tile
from concourse.masks import make_identity

import ml_dtypes

BF16NP = ml_dtypes.bfloat16

F32 = mybir.dt.float32
BF16 = mybir.dt.bfloat16
FP16 = mybir.dt.float16
I32 = mybir.dt.int32

N, E, G, H = 50000, 800000, 128, 128
NC = 8
NPC = N // NC            # 6250 nodes per core
NT = 49                  # node tiles per core (48*128 + 106)
ENT = 2304               # padded edge slots per node tile (18 subchunks)
SNT = ENT // 128         # 18 subchunks of 128 edges
# FM chunk plan: (subchunk offset j0, subchunk count S)
CHUNKS = [(0, 4), (4, 4), (8, 4), (12, 4), (16, 2)]
NCHUNK = len(CHUNKS)
EPS = 1e-5
AF = mybir.ActivationFunctionType
OP = mybir.AluOpType


# --------------------------------------------------------------------------
# host-side prep: pure index manipulation / padding / layout
# --------------------------------------------------------------------------

def _host_prep(inputs):
    src = np.asarray(inputs["edge_index"][0]).astype(np.int32)
    dst = np.asarray(inputs["edge_index"][1]).astype(np.int32)
    e2g = np.asarray(inputs["edge2graph"]).astype(np.int32)
    fr = np.asarray(inputs["frac_coords"], np.float32)
    deg = np.bincount(src, minlength=N)
    perm = np.argsort(src, kind="stable")
    srcS, dstS, e2gS = src[perm], dst[perm], e2g[perm]

    # mod-1 wrap bits per edge (exact, from f32 coords)
    d3 = fr[dstS] - fr[srcS]
    kcode = ((d3[:, 0] < 0).astype(np.int32)
             + 2 * (d3[:, 1] < 0).astype(np.int32)
             + 4 * (d3[:, 2] < 0).astype(np.int32))
    cidxS = kcode * G + e2gS
    degS = np.maximum(deg[srcS], 1).astype(np.float32)

    dstT = np.zeros((NC * NT, 128, SNT), np.int32)
    cidxT = np.zeros((NC * NT, 128, SNT), np.int32)
    srclT = np.full((NC * NT, 128, SNT), 200.0, np.float16)
    degT = np.ones((NC * NT, 128, SNT), np.float16)
    stairsT = np.zeros((NC * NT, 128, 2 * NCHUNK), np.float16)

    for c in range(NC):
        m0 = c * NPC
        lo = np.searchsorted(srcS, m0)
        hi = np.searchsorted(srcS, m0 + NPC)
        s_c = srcS[lo:hi]
        d_c, g_c, dg_c = dstS[lo:hi], cidxS[lo:hi], degS[lo:hi]
        nb = np.searchsorted(s_c, m0 + 128 * np.arange(NT + 1))
        for nt in range(NT):
            a, b = nb[nt], nb[nt + 1]
            cnt = b - a
            assert cnt <= ENT, f"node tile overflow: {cnt} > {ENT}"
            sl = s_c[a:b]
            row = c * NT + nt

            def pack(vals, fill, dtype):
                buf = np.full(ENT, fill, dtype)
                buf[:cnt] = vals
                return buf.reshape(SNT, 128).T  # [128, SNT]

            dstT[row] = pack(d_c[a:b], 0, np.int32)
            cidxT[row] = pack(g_c[a:b], 0, np.int32)
            srclT[row] = pack((sl - (m0 + 128 * nt)).astype(np.float16),
                              200.0, np.float16)
            degT[row] = pack(dg_c[a:b].astype(np.float16), 1.0, np.float16)

            node_ids = m0 + 128 * nt + np.arange(128)
            st = np.searchsorted(sl, node_ids)
            en = np.searchsorted(sl, node_ids + 1)
            for ci, (j0, S) in enumerate(CHUNKS):
                base, w = j0 * 128, S * 128
                stairsT[row, :, 2 * ci] = np.clip(st - base, 0, w)
                stairsT[row, :, 2 * ci + 1] = np.clip(en - base, 0, w)

    # frac transposed per own-node tile: frT[c*NT+nt, comp, p] = fr[node, comp]
    frT = np.zeros((NC * NT, 4, 128), np.float32)
    pidx = np.arange(NT * 128)
    valid = pidx < NPC
    for c in range(NC):
        f = np.zeros((NT * 128, 3), np.float32)
        f[valid] = fr[c * NPC + pidx[valid]]
        frT[c * NT:(c + 1) * NT, :3, :] = f.reshape(NT, 128, 3).transpose(0, 2, 1)

    return dict(dstT=dstT, cidxT=cidxT, srclT=srclT, degT=degT,
                stairsT=stairsT, frT=frT.astype(BF16NP))


def _host_weights(inputs):
    gam = np.asarray(inputs["gamma"], np.float32)
    bet = np.asarray(inputs["beta"], np.float32)
    We1 = np.asarray(inputs["We1"], np.float32)
    Wa, Wb = We1[0:128], We1[128:256]
    Wl, Wf = We1[256:265], We1[265:268]
    be1tot = np.asarray(inputs["be1"], np.float32) + bet @ (Wa + Wb)

    lat = np.asarray(inputs["lattices"], np.float32)
    lat_ip = np.einsum("gij,gkj->gik", lat, lat).reshape(G, 9)
    wlat = lat_ip @ Wl + be1tot  # [G, H]
    kmat = np.array([[(b >> c) & 1 for c in range(3)] for b in range(8)],
                    np.float32)
    kWf = kmat @ Wf  # [8, H]
    comb = (wlat[None, :, :] + kWf[:, None, :]).reshape(8 * G, H)

    def pad4(w):
        out = np.zeros((4, H), np.float32)
        out[:3] = w
        return out

    Wn1 = np.asarray(inputs["Wn1"], np.float32)
    Wn1h, Wn1a = Wn1[0:128], Wn1[128:256]
    bn1tot = np.asarray(inputs["bn1"], np.float32) + bet @ Wn1h

    return dict(
        Wap=(gam[:, None] * Wa).astype(BF16NP),
        Wbp=(gam[:, None] * Wb).astype(BF16NP),
        Wfp=pad4(Wf).astype(BF16NP),
        Wfn=pad4(-Wf).astype(BF16NP),
        comb=comb.astype(BF16NP),
        We2b=np.asarray(inputs["We2"], np.float32).astype(BF16NP),
        be2=np.asarray(inputs["be2"], np.float32),
        Wn1hb=(gam[:, None] * Wn1h).astype(BF16NP),
        Wn1ab=Wn1a.astype(BF16NP),
        bn1t=bn1tot,
        Wn2b=np.asarray(inputs["Wn2"], np.float32).astype(BF16NP),
        bn2=np.asarray(inputs["bn2"], np.float32),
    )


# --------------------------------------------------------------------------
# bass program (single SPMD program for all 8 cores)
# --------------------------------------------------------------------------

def build_program():
    nc = bacc.Bacc()
    p = lambda n, s, d: nc.declare_dram_parameter(n, list(s), d, isOutput=False)

    xown = p("xown", (NPC, H), F32)
    frT = p("frT", (NT, 4, 128), BF16)
    dstT = p("dstT", (NT, 128, SNT), I32)
    cidxT = p("cidxT", (NT, 128, SNT), I32)
    srclT = p("srclT", (NT, 128, SNT), FP16)
    degT = p("degT", (NT, 128, SNT), FP16)
    stairsT = p("stairsT", (NT, 128, 2 * NCHUNK), FP16)
    comb = p("comb", (8 * G, H), BF16)
    Wap = p("Wap", (H, H), BF16)
    Wbp = p("Wbp", (H, H), BF16)
    Wfp = p("Wfp", (4, H), BF16)
    Wfn = p("Wfn", (4, H), BF16)
    We2b = p("We2b", (H, H), BF16)
    Wn1hb = p("Wn1hb", (H, H), BF16)
    Wn1ab = p("Wn1ab", (H, H), BF16)
    Wn2b = p("Wn2b", (H, H), BF16)
    be2 = p("be2", (H,), F32)
    bn1t = p("bn1t", (H,), F32)
    bn2 = p("bn2", (H,), F32)

    out = nc.declare_dram_parameter("nout", [NPC, H], BF16, isOutput=True)

    with tile.TileContext(nc) as tc:
        with (
            tc.tile_pool(name="dram", bufs=1, space="DRAM") as dram,
            tc.tile_pool(name="persist", bufs=1) as pp,
        ):
            zbslice = dram.tile([NPC, H], BF16)
            zb_tbl = dram.tile([N, H], BF16)

            # ---------------- constants ----------------
            I_bf = pp.tile([128, 128], BF16)
            make_identity(nc, I_bf[:])
            iota_i = pp.tile([128, 512], I32)
            nc.gpsimd.iota(iota_i[:], pattern=[[1, 512]], base=0,
                           channel_multiplier=0)
            iota_f = pp.tile([128, 512], F32)
            nc.any.tensor_copy(out=iota_f[:], in_=iota_i[:])

            def load_col(ap, tag):
                t = pp.tile([128, 1], F32, tag=tag)
                nc.sync.dma_start(out=t[:], in_=ap[:, None])
                return t

            be2c = load_col(be2, "be2c")
            bn1c = load_col(bn1t, "bn1c")
            bn2c = load_col(bn2, "bn2c")
            epsc = pp.tile([128, 1], F32)
            nc.gpsimd.memset(epsc[:], EPS)

            def load_w(ap, shape, tag):
                t = pp.tile(list(shape), BF16, tag=tag)
                nc.sync.dma_start(out=t[:], in_=ap[:, :])
                return t

            Wap_s = load_w(Wap, (128, 128), "Wap_s")
            Wbp_s = load_w(Wbp, (128, 128), "Wbp_s")
            Wfp_s = load_w(Wfp, (4, 128), "Wfp_s")
            Wfn_s = load_w(Wfn, (4, 128), "Wfn_s")
            We2_s = load_w(We2b, (128, 128), "We2_s")
            Wn1h_s = load_w(Wn1hb, (128, 128), "Wn1h_s")
            Wn1a_s = load_w(Wn1ab, (128, 128), "Wn1a_s")
            Wn2_s = load_w(Wn2b, (128, 128), "Wn2_s")

            # persistent per-core state
            za_own = pp.tile([128, NT, 128], BF16)
            h0T_own = pp.tile([128, NT, 128], BF16)
            nc.gpsimd.memset(za_own[:], 0.0)
            nc.gpsimd.memset(h0T_own[:], 0.0)

            # ---- phase 1: own nodes -> h0T_own, za_own, zbslice ----
            with (
                tc.tile_pool(name="p1", bufs=3) as pl,
                tc.tile_pool(name="p1psT", bufs=2, space="PSUM") as pps,
                tc.tile_pool(name="p1psZ", bufs=2, space="PSUM") as pps1,
            ):
                for nt in range(NT):
                    rows = 106 if nt == NT - 1 else 128
                    xt = pl.tile([128, 128], F32, tag="xt")
                    nc.sync.dma_start(out=xt[:rows, :],
                                      in_=xown[nt * 128:nt * 128 + rows, :])
                    frt = pl.tile([4, 128], BF16, tag="frt")
                    nc.sync.dma_start(out=frt[:], in_=frT[nt, :, :])
                    st6 = pl.tile([128, 6], F32, tag="st6")
                    nc.vector.bn_stats(st6[:rows, :], xt[:rows, :])
                    st2 = pl.tile([128, 2], F32, tag="st2")
                    nc.vector.bn_aggr(st2[:rows, :], st6[:rows, :])
                    sd = pl.tile([128, 1], F32, tag="sd")
                    nc.scalar.activation(sd[:rows, :], st2[:rows, 1:2],
                                         AF.Sqrt, bias=epsc[:rows, :])
                    a = pl.tile([128, 1], F32, tag="a")
                    nc.vector.reciprocal(a[:rows, :], sd[:rows, :])
                    bnn = pl.tile([128, 1], F32, tag="bnn")
                    nc.vector.tensor_scalar(bnn[:rows, :], st2[:rows, 0:1],
                                            a[:rows, :], -1.0, OP.mult, OP.mult)
                    h0 = pl.tile([128, 128], BF16, tag="h0")
                    nc.scalar.activation(h0[:rows, :], xt[:rows, :],
                                         AF.Identity, bias=bnn[:rows, :],
                                         scale=a[:rows, :])
                    ps_t = pps.tile([128, 128], BF16, tag="psT")
                    nc.tensor.matmul(ps_t[:, :rows], h0[:rows, :],
                                     I_bf[:rows, :rows],
                                     is_transpose=True, start=True, stop=True)
                    nc.any.tensor_copy(out=h0T_own[:, nt, :rows],
                                       in_=ps_t[:, :rows])
                    ps_za = pps1.tile([128, 128], F32, tag="psza")
                    nc.tensor.matmul(ps_za[:], lhsT=h0T_own[:, nt, :],
                                     rhs=Wap_s[:], start=True, stop=False,
                                     skip_group_check=True)
                    nc.tensor.matmul(ps_za[:], lhsT=frt[:], rhs=Wfn_s[:],
                                     start=False, stop=True,
                                     skip_group_check=True)
                    nc.any.tensor_copy(out=za_own[:, nt, :], in_=ps_za[:])
                    ps_zb = pps1.tile([128, 128], F32, tag="pszb")
                    nc.tensor.matmul(ps_zb[:], lhsT=h0T_own[:, nt, :],
                                     rhs=Wbp_s[:], start=True, stop=False,
                                     skip_group_check=True)
                    nc.tensor.matmul(ps_zb[:], lhsT=frt[:], rhs=Wfp_s[:],
                                     start=False, stop=True,
                                     skip_group_check=True)
                    zbb = pl.tile([128, 128], BF16, tag="zbb")
                    nc.any.tensor_copy(out=zbb[:], in_=ps_zb[:])
                    nc.sync.dma_start(out=zbslice[nt * 128:nt * 128 + rows, :],
                                      in_=zbb[:rows, :])

            # ---- share zb across cores ----
            nc.gpsimd.collective_compute(
                "AllGather", OP.bypass,
                replica_groups=[list(range(NC))],
                ins=[zbslice[:].opt()],
                outs=[zb_tbl[:].opt()],
            )

            # ---------------- phase 2: edges + node update ----------------
            with (
                tc.tile_pool(name="idx", bufs=2) as pidx,
                tc.tile_pool(name="gat", bufs=2) as pg,
                tc.tile_pool(name="work", bufs=2) as pw,
                tc.tile_pool(name="ps_z1", bufs=2, space="PSUM") as ps_z1,
                tc.tile_pool(name="ps_z2", bufs=2, space="PSUM") as ps_z2,
                tc.tile_pool(name="ps_agg", bufs=2, space="PSUM") as ps_agg,
            ):
                for nt in range(NT):
                    rows = 106 if nt == NT - 1 else 128
                    # ---- index loads ----
                    t_dst = pidx.tile([128, SNT], I32, tag="dst")
                    nc.sync.dma_start(out=t_dst[:], in_=dstT[nt, :, :])
                    t_cid = pidx.tile([128, SNT], I32, tag="cid")
                    nc.sync.dma_start(out=t_cid[:], in_=cidxT[nt, :, :])
                    t_srl16 = pidx.tile([128, SNT], FP16, tag="srl16")
                    nc.sync.dma_start(out=t_srl16[:], in_=srclT[nt, :, :])
                    t_deg16 = pidx.tile([128, SNT], FP16, tag="deg16")
                    nc.sync.dma_start(out=t_deg16[:], in_=degT[nt, :, :])
                    t_str16 = pidx.tile([128, 2 * NCHUNK], FP16, tag="str16")
                    nc.sync.dma_start(out=t_str16[:], in_=stairsT[nt, :, :])

                    t_srl = pidx.tile([128, SNT], F32, tag="srl")
                    nc.any.tensor_copy(out=t_srl[:], in_=t_srl16[:])
                    t_degf = pidx.tile([128, SNT], F32, tag="degf")
                    nc.any.tensor_copy(out=t_degf[:], in_=t_deg16[:])
                    t_inv = pidx.tile([128, SNT], F32, tag="inv")
                    nc.vector.reciprocal(t_inv[:], t_degf[:])
                    t_str = pidx.tile([128, 2 * NCHUNK], F32, tag="str")
                    nc.any.tensor_copy(out=t_str[:], in_=t_str16[:])

                    # ---- gathers (edge-major, one row per partition) ----
                    g_zb = pg.tile([128, SNT, 128], BF16, tag="gzb")
                    g_cb = pg.tile([128, SNT, 128], BF16, tag="gcb")
                    for j in range(SNT):
                        nc.gpsimd.indirect_dma_start(
                            out=g_zb[:, j, :], out_offset=None,
                            in_=zb_tbl[:, :],
                            in_offset=bass.IndirectOffsetOnAxis(
                                ap=t_dst[:, j:j + 1], axis=0))
                        nc.gpsimd.indirect_dma_start(
                            out=g_cb[:, j, :], out_offset=None,
                            in_=comb[:, :],
                            in_offset=bass.IndirectOffsetOnAxis(
                                ap=t_cid[:, j:j + 1], axis=0))

                    agg = ps_agg.tile([128, 128], F32, tag="agg")

                    for ci, (j0, S) in enumerate(CHUNKS):
                        W = S * 128
                        # staircase selection matrix selT [128n, W]
                        t0 = pw.tile([128, 512], BF16, tag="t0")
                        nc.vector.tensor_scalar(
                            t0[:, :W], iota_f[:, :W],
                            t_str[:, 2 * ci + 1:2 * ci + 2], None, OP.is_lt)
                        selT = pw.tile([128, 512], BF16, tag="selT")
                        nc.vector.scalar_tensor_tensor(
                            out=selT[:, :W], in0=iota_f[:, :W],
                            scalar=t_str[:, 2 * ci:2 * ci + 1],
                            in1=t0[:, :W], op0=OP.is_ge, op1=OP.mult)

                        # zb + comb summed, then xbar-transposed to FM
                        gsum = pw.tile([128, 4, 128], BF16, tag="gsum")
                        nc.vector.tensor_tensor(
                            out=gsum[:, :S, :], in0=g_zb[:, j0:j0 + S, :],
                            in1=g_cb[:, j0:j0 + S, :], op=OP.add)
                        gT = pw.tile([128, 4, 128], BF16, tag="gT")
                        nc.sync.dma_start_transpose(gT[:, :S, :],
                                                    gsum[:, :S, :])

                        # z1T accumulation [128H, W]
                        z1 = ps_z1.tile([128, 512], F32, tag="z1")
                        nc.tensor.matmul(z1[:, :W], lhsT=za_own[:, nt, :],
                                         rhs=selT[:, :W], start=True,
                                         stop=False, skip_group_check=True)
                        nc.tensor.matmul(z1[:, :W], lhsT=I_bf[:],
                                         rhs=gT[:, :S, :], start=False,
                                         stop=True, skip_group_check=True)

                        e1T = pw.tile([128, 512], BF16, tag="e1T")
                        nc.scalar.activation(e1T[:, :W], z1[:, :W], AF.Silu)

                        z2 = ps_z2.tile([128, 512], F32, tag="z2")
                        nc.tensor.matmul(z2[:, :W], lhsT=We2_s[:],
                                         rhs=e1T[:, :W], start=True, stop=True)
                        e2T = pw.tile([128, 512], BF16, tag="e2T")
                        nc.scalar.activation(e2T[:, :W], z2[:, :W], AF.Silu,
                                             bias=be2c[:])
                        e2em = pw.tile([128, 4, 128], BF16, tag="e2em")
                        nc.sync.dma_start_transpose(e2em[:, :S, :], e2T[:, :W])

                        # scatter-mean matmuls into agg [128H, 128n]
                        for j in range(S):
                            jj = j0 + j
                            selp = pw.tile([128, 128], BF16, tag="selp")
                            nc.vector.tensor_scalar(
                                selp[:], iota_f[:, :128],
                                t_srl[:, jj:jj + 1], t_inv[:, jj:jj + 1],
                                OP.is_equal, OP.mult)
                            nc.tensor.matmul(
                                agg[:], lhsT=e2em[:, j, :], rhs=selp[:],
                                start=(ci == 0 and j == 0),
                                stop=(ci == NCHUNK - 1 and j == S - 1),
                                skip_group_check=True)

                    # ---- node update for this tile ----
                    aggb = pw.tile([128, 128], BF16, tag="aggb")
                    nc.any.tensor_copy(out=aggb[:], in_=agg[:])
                    n1 = ps_z1.tile([128, 512], F32, tag="z1")
                    nc.tensor.matmul(n1[:, :128], lhsT=Wn1h_s[:],
                                     rhs=h0T_own[:, nt, :], start=True,
                                     stop=False, skip_group_check=True)
                    nc.tensor.matmul(n1[:, :128], lhsT=Wn1a_s[:], rhs=aggb[:],
                                     start=False, stop=True,
                                     skip_group_check=True)
                    n1T = pw.tile([128, 128], BF16, tag="n1T")
                    nc.scalar.activation(n1T[:], n1[:, :128], AF.Silu,
                                         bias=bn1c[:])
                    n2 = ps_z2.tile([128, 512], F32, tag="z2")
                    nc.tensor.matmul(n2[:, :128], lhsT=Wn2_s[:], rhs=n1T[:],
                                     start=True, stop=True)
                    n2T = pw.tile([128, 128], BF16, tag="n2T")
                    nc.scalar.activation(n2T[:], n2[:, :128], AF.Silu,
                                         bias=bn2c[:])
                    n2em = pw.tile([128, 1, 128], BF16, tag="n2em")
                    nc.sync.dma_start_transpose(n2em[:], n2T[:])
                    nc.sync.dma_start(out=out[nt * 128:nt * 128 + rows, :],
                                      in_=n2em[:rows, 0, :])
    nc.finalize()
    return nc


# --------------------------------------------------------------------------
# cached jit runner (trace/lower/compile once per process)
# --------------------------------------------------------------------------

class _Result:
    exec_time_ns = None
    profile_json = None
    mean_exec_time_ns = None
    results = None


class _Runner:
    def __init__(self):
        import jax
        import jax.numpy as jnp
        from jax.sharding import Mesh, PartitionSpec, NamedSharding
        from jax.experimental.shard_map import shard_map
        from concourse.bass2jax import (
            _bass_exec_p, install_neuronx_cc_hook, partition_id_tensor)

        self.jax = jax
        nc = build_program()
        self.nc = nc
        install_neuronx_cc_hook()

        partition_name = (nc.partition_id_tensor.name
                          if nc.partition_id_tensor else None)
        in_names, out_names, out_avals = [], [], []
        for alloc in nc.m.functions[0].allocations:
            if not isinstance(alloc, mybir.MemoryLocationSet):
                continue
            name = alloc.memorylocations[0].name
            if alloc.kind == "ExternalInput":
                if name != partition_name:
                    in_names.append(name)
            elif alloc.kind == "ExternalOutput":
                out_names.append(name)
                out_avals.append(jax.core.ShapedArray(
                    tuple(alloc.tensor_shape), mybir.dt.np(alloc.dtype)))
        self.in_names, self.out_names = in_names, out_names
        n_params, n_outs = len(in_names), len(out_avals)
        all_in = tuple(in_names + out_names
                       + ([partition_name] if partition_name else []))

        def _body(*args):
            operands = list(args)
            if partition_name is not None:
                operands.append(partition_id_tensor())
            outs = _bass_exec_p.bind(
                *operands, out_avals=tuple(out_avals), in_names=all_in,
                out_names=tuple(out_names), lowering_input_output_aliases=(),
                sim_require_finite=True, sim_require_nnan=True, nc=nc)
            return tuple(outs)

        devices = jax.devices()[:NC]
        assert len(devices) == NC
        mesh = Mesh(np.asarray(devices), ("core",))
        PS = PartitionSpec
        donate = tuple(range(n_params, n_params + n_outs))
        self.fn = jax.jit(
            shard_map(_body, mesh=mesh,
                      in_specs=(PS("core"),) * (n_params + n_outs),
                      out_specs=(PS("core"),) * n_outs, check_rep=False),
            donate_argnums=donate, keep_unused=True)

        sh = NamedSharding(mesh, PS("core"))
        zshapes = [(NC * a.shape[0], *a.shape[1:]) for a in out_avals]
        zdtypes = [a.dtype for a in out_avals]
        self.make_zeros = jax.jit(
            lambda: tuple(jnp.zeros(s, d) for s, d in zip(zshapes, zdtypes)),
            out_shardings=(sh,) * n_outs)

    def __call__(self, arg_map):
        args = [arg_map[n] for n in self.in_names]
        zeros = self.make_zeros()
        outs = self.fn(*args, *zeros)
        return {name: outs[i] for i, name in enumerate(self.out_names)}


_RUNNER = None


def kernel(**inputs) -> np.ndarray:
    out, _ = run(inputs, trace=False)
    return out


def run(inputs, trace=False):
    global _RUNNER
    if _RUNNER is None:
        _RUNNER = _Runner()

    x = np.ascontiguousarray(np.asarray(inputs["node_features"], np.float32))
    idx = _host_prep(inputs)
    wts = _host_weights(inputs)

    am = dict(xown=x)
    am.update(idx)
    for k, v in wts.items():
        # replicate per core along axis 0
        am[k] = np.tile(v, (NC,) + (1,) * (v.ndim - 1)) if v.ndim > 1 \
            else np.tile(v, NC)

    outs = _RUNNER(am)
    nout = np.asarray(outs["nout"])  # [N, H] bf16, core-order == node-order
    res = _Result()
    return x + nout.astype(np.float32), res


if __name__ == "__main__":
    build_program()
    print("program built OK")


# revision 8
# speedup vs baseline: 7.2231x; 1.1076x over previous
"""Trainium2 Bass kernel for nn_CSPLayer (GNN message passing), 8 NeuronCores.

Strategy: sort edges by src node; core c owns nodes [c*6250,(c+1)*6250) and all
their outgoing edges (scatter over src is then core-local). Per core the edges
are grouped by 128-node tiles, each padded to a fixed 2304 slots so every core
runs an identical instruction stream (SPMD).

v2 pipeline changes vs v1:
  - x is sharded (each core receives only its own 6250 rows); every core
    computes the zb table for its own nodes and an on-device AllGather
    builds the full [N,H] zb table each core gathers from.  This removes
    the 8x-replicated 25.6MB x input (205MB -> 25.6MB host->device).
  - frac terms are folded into per-node tables: frac_diff = fj - fi + k
    with k in {0,1}^3 the mod-1 wrap bits (computed exactly on host).
    fj@Wf folds into zb, -fi@Wf into za, and k@Wf + lat_ip@Wl + be1tot
    into a 1024-row combined table indexed by (k*128 + graph).  This
    eliminates the frac gathers and all per-edge frac math.
  - gamma/beta/biases folded into bf16 weights on the host.
  - jit runner is cached module-wide (trace/lower/compile once); donated
    output zero-buffers are created on device; output is n (bf16), the
    residual x + n is added on the host in f32.

Math:
  h   = LN(x);  h0 = (x-mu)*rsqrt(var+eps)   (gamma/beta folded into weights)
  za  = h0 @ (gamma*Wa) - frac @ Wf          (own nodes, SBUF resident, bf16)
  zb  = h0 @ (gamma*Wb) + frac @ Wf          (own slice -> AllGather -> [N,H])
  comb[k*128+g] = (L L^T)[g] @ Wl + be1 + beta@(Wa+Wb) + k @ Wf
  z1T[:,e] = za[src] (stair-matmul) + zb[dst]^T + comb[kcode,e2g]^T
  e1 = silu(z1); e2 = silu(e1@We2+be2); agg = scatter-mean over src
  n  = silu(silu([h|agg]@Wn1+bn1)@Wn2+bn2);  out = x + n (host add)
"""

import os
import sys

import numpy as np

if "/opt/trn_rl_repo" not in sys.path:
    sys.path.insert(0, "/opt/trn_rl_repo")

import concourse.bass as bass
import concourse.bacc as bacc
import concourse.mybir as mybir
import concourse.tile as tile
from concourse.masks import make_identity

import ml_dtypes

BF16NP = ml_dtypes.bfloat16

F32 = mybir.dt.float32
BF16 = mybir.dt.bfloat16
FP16 = mybir.dt.float16
I32 = mybir.dt.int32

N, E, G, H = 50000, 800000, 128, 128
NC = 8
NPC = N // NC            # 6250 nodes per core
NT = 49                  # node tiles per core (48*128 + 106)
ENT = 2304               # padded edge slots per node tile (18 subchunks)
SNT = ENT // 128         # 18 subchunks of 128 edges
# FM chunk plan: (subchunk offset j0, subchunk count S)
CHUNKS = [(0, 4), (4, 4), (8, 4), (12, 4), (16, 2)]
NCHUNK = len(CHUNKS)
EPS = 1e-5
AF = mybir.ActivationFunctionType
OP = mybir.AluOpType


# --------------------------------------------------------------------------
# host-side prep: pure index manipulation / padding / layout
# --------------------------------------------------------------------------

def _host_prep(inputs):
    src = np.asarray(inputs["edge_index"][0]).astype(np.int32)
    dst = np.asarray(inputs["edge_index"][1]).astype(np.int32)
    e2g = np.asarray(inputs["edge2graph"]).astype(np.int32)
    fr = np.asarray(inputs["frac_coords"], np.float32)
    deg = np.bincount(src, minlength=N)
    perm = np.argsort(src, kind="stable")
    srcS, dstS, e2gS = src[perm], dst[perm], e2g[perm]

    # mod-1 wrap bits per edge (exact, from f32 coords)
    d3 = fr[dstS] - fr[srcS]
    kcode = ((d3[:, 0] < 0).astype(np.int32)
             + 2 * (d3[:, 1] < 0).astype(np.int32)
             + 4 * (d3[:, 2] < 0).astype(np.int32))
    cidxS = kcode * G + e2gS

    # edge -> (tile row, slot) fully vectorized: edges sorted by src are
    # contiguous per 128-node tile; slot = edge rank within its tile.
    cum = np.zeros(N + 1, np.int64)
    np.cumsum(deg, out=cum[1:])
    c_of = srcS // NPC
    loc = srcS - c_of * NPC
    nt_of = loc >> 7
    row_of = c_of * NT + nt_of
    first_node = c_of * NPC + (nt_of << 7)
    slot = np.arange(E, dtype=np.int64) - cum[first_node]
    assert slot.max() < ENT, f"node tile overflow: {slot.max() + 1} > {ENT}"
    part = (slot & 127).astype(np.int64)
    col = (slot >> 7).astype(np.int64)

    dstT = np.zeros((NC * NT, 128, SNT), np.int32)
    dstT[row_of, part, col] = dstS
    cidxT = np.zeros((NC * NT, 128, SNT), np.int32)
    cidxT[row_of, part, col] = cidxS
    srclT = np.full((NC * NT, 128, SNT), 200.0, np.float16)
    srclT[row_of, part, col] = (loc & 127).astype(np.float16)
    degT = np.ones((NC * NT, 128, SNT), np.float16)
    degT[row_of, part, col] = np.maximum(deg[srcS], 1).astype(np.float16)

    # stair boundaries per (core, tile, node): global edge offsets from cum
    off = np.arange(NT * 128)
    coreb = (np.arange(NC) * NPC)[:, None]
    stid = np.minimum(off, NPC)[None, :] + coreb
    enid = np.minimum(off + 1, NPC)[None, :] + coreb
    base = cum[coreb[:, :, None] + (np.arange(NT) * 128)[None, None, :]
               ].reshape(NC, NT, 1)
    st = cum[stid].reshape(NC, NT, 128) - base
    en = cum[enid].reshape(NC, NT, 128) - base
    stairs = np.empty((NC, NT, 128, 2 * NCHUNK), np.float16)
    for ci, (j0, S) in enumerate(CHUNKS):
        b, w = j0 * 128, S * 128
        stairs[..., 2 * ci] = np.clip(st - b, 0, w)
        stairs[..., 2 * ci + 1] = np.clip(en - b, 0, w)
    stairsT = stairs.reshape(NC * NT, 128, 2 * NCHUNK)

    # frac transposed per own-node tile: frT[c*NT+nt, comp, p] = fr[node, comp]
    frT = np.zeros((NC, NT * 128, 4), np.float32)
    valid = np.arange(NT * 128) < NPC
    frT[:, valid, :3] = fr.reshape(NC, NPC, 3)
    frT = np.ascontiguousarray(
        frT.reshape(NC * NT, 128, 4).transpose(0, 2, 1)).astype(BF16NP)

    return dict(dstT=dstT, cidxT=cidxT, srclT=srclT, degT=degT,
                stairsT=stairsT, frT=frT)


def _host_weights(inputs):
    gam = np.asarray(inputs["gamma"], np.float32)
    bet = np.asarray(inputs["beta"], np.float32)
    We1 = np.asarray(inputs["We1"], np.float32)
    Wa, Wb = We1[0:128], We1[128:256]
    Wl, Wf = We1[256:265], We1[265:268]
    be1tot = np.asarray(inputs["be1"], np.float32) + bet @ (Wa + Wb)

    lat = np.asarray(inputs["lattices"], np.float32)
    lat_ip = np.einsum("gij,gkj->gik", lat, lat).reshape(G, 9)
    wlat = lat_ip @ Wl + be1tot  # [G, H]
    kmat = np.array([[(b >> c) & 1 for c in range(3)] for b in range(8)],
                    np.float32)
    kWf = kmat @ Wf  # [8, H]
    comb = (wlat[None, :, :] + kWf[:, None, :]).reshape(8 * G, H)

    def pad4(w):
        out = np.zeros((4, H), np.float32)
        out[:3] = w
        return out

    Wn1 = np.asarray(inputs["Wn1"], np.float32)
    Wn1h, Wn1a = Wn1[0:128], Wn1[128:256]
    bn1tot = np.asarray(inputs["bn1"], np.float32) + bet @ Wn1h

    return dict(
        Wap=(gam[:, None] * Wa).astype(BF16NP),
        Wbp=(gam[:, None] * Wb).astype(BF16NP),
        Wfp=pad4(Wf).astype(BF16NP),
        Wfn=pad4(-Wf).astype(BF16NP),
        comb=comb.astype(BF16NP),
        We2b=np.asarray(inputs["We2"], np.float32).astype(BF16NP),
        be2=np.asarray(inputs["be2"], np.float32),
        Wn1hb=(gam[:, None] * Wn1h).astype(BF16NP),
        Wn1ab=Wn1a.astype(BF16NP),
        bn1t=bn1tot,
        Wn2b=np.asarray(inputs["Wn2"], np.float32).astype(BF16NP),
        bn2=np.asarray(inputs["bn2"], np.float32),
    )


# --------------------------------------------------------------------------
# bass program (single SPMD program for all 8 cores)
# --------------------------------------------------------------------------

def build_program():
    nc = bacc.Bacc()
    p = lambda n, s, d: nc.declare_dram_parameter(n, list(s), d, isOutput=False)

    xown = p("xown", (NPC, H), BF16)
    frT = p("frT", (NT, 4, 128), BF16)
    dstT = p("dstT", (NT, 128, SNT), I32)
    cidxT = p("cidxT", (NT, 128, SNT), I32)
    srclT = p("srclT", (NT, 128, SNT), FP16)
    degT = p("degT", (NT, 128, SNT), FP16)
    stairsT = p("stairsT", (NT, 128, 2 * NCHUNK), FP16)
    comb = p("comb", (8 * G, H), BF16)
    Wap = p("Wap", (H, H), BF16)
    Wbp = p("Wbp", (H, H), BF16)
    Wfp = p("Wfp", (4, H), BF16)
    Wfn = p("Wfn", (4, H), BF16)
    We2b = p("We2b", (H, H), BF16)
    Wn1hb = p("Wn1hb", (H, H), BF16)
    Wn1ab = p("Wn1ab", (H, H), BF16)
    Wn2b = p("Wn2b", (H, H), BF16)
    be2 = p("be2", (H,), F32)
    bn1t = p("bn1t", (H,), F32)
    bn2 = p("bn2", (H,), F32)

    out = nc.declare_dram_parameter("nout", [NPC, H], BF16, isOutput=True)

    with tile.TileContext(nc) as tc:
        with (
            tc.tile_pool(name="dram", bufs=1, space="DRAM") as dram,
            tc.tile_pool(name="persist", bufs=1) as pp,
        ):
            zbslice = dram.tile([NPC, H], BF16)
            zb_tbl = dram.tile([N, H], BF16)

            # ---------------- constants ----------------
            I_bf = pp.tile([128, 128], BF16)
            make_identity(nc, I_bf[:])
            iota_i = pp.tile([128, 512], I32)
            nc.gpsimd.iota(iota_i[:], pattern=[[1, 512]], base=0,
                           channel_multiplier=0)
            iota_f = pp.tile([128, 512], F32)
            nc.any.tensor_copy(out=iota_f[:], in_=iota_i[:])

            def load_col(ap, tag):
                t = pp.tile([128, 1], F32, tag=tag)
                nc.sync.dma_start(out=t[:], in_=ap[:, None])
                return t

            be2c = load_col(be2, "be2c")
            bn1c = load_col(bn1t, "bn1c")
            bn2c = load_col(bn2, "bn2c")
            epsc = pp.tile([128, 1], F32)
            nc.gpsimd.memset(epsc[:], EPS)

            def load_w(ap, shape, tag):
                t = pp.tile(list(shape), BF16, tag=tag)
                nc.sync.dma_start(out=t[:], in_=ap[:, :])
                return t

            Wap_s = load_w(Wap, (128, 128), "Wap_s")
            Wbp_s = load_w(Wbp, (128, 128), "Wbp_s")
            Wfp_s = load_w(Wfp, (4, 128), "Wfp_s")
            Wfn_s = load_w(Wfn, (4, 128), "Wfn_s")
            We2_s = load_w(We2b, (128, 128), "We2_s")
            Wn1h_s = load_w(Wn1hb, (128, 128), "Wn1h_s")
            Wn1a_s = load_w(Wn1ab, (128, 128), "Wn1a_s")
            Wn2_s = load_w(Wn2b, (128, 128), "Wn2_s")

            # persistent per-core state
            za_own = pp.tile([128, NT, 128], BF16)
            h0T_own = pp.tile([128, NT, 128], BF16)
            nc.gpsimd.memset(za_own[:], 0.0)
            nc.gpsimd.memset(h0T_own[:], 0.0)

            # ---- phase 1: own nodes -> h0T_own, za_own, zbslice ----
            with (
                tc.tile_pool(name="p1", bufs=3) as pl,
                tc.tile_pool(name="p1psT", bufs=2, space="PSUM") as pps,
                tc.tile_pool(name="p1psZ", bufs=2, space="PSUM") as pps1,
            ):
                for nt in range(NT):
                    rows = 106 if nt == NT - 1 else 128
                    xt_b = pl.tile([128, 128], BF16, tag="xtb")
                    nc.sync.dma_start(out=xt_b[:rows, :],
                                      in_=xown[nt * 128:nt * 128 + rows, :])
                    xt = pl.tile([128, 128], F32, tag="xt")
                    nc.any.tensor_copy(out=xt[:rows, :], in_=xt_b[:rows, :])
                    frt = pl.tile([4, 128], BF16, tag="frt")
                    nc.sync.dma_start(out=frt[:], in_=frT[nt, :, :])
                    st6 = pl.tile([128, 6], F32, tag="st6")
                    nc.vector.bn_stats(st6[:rows, :], xt[:rows, :])
                    st2 = pl.tile([128, 2], F32, tag="st2")
                    nc.vector.bn_aggr(st2[:rows, :], st6[:rows, :])
                    sd = pl.tile([128, 1], F32, tag="sd")
                    nc.scalar.activation(sd[:rows, :], st2[:rows, 1:2],
                                         AF.Sqrt, bias=epsc[:rows, :])
                    a = pl.tile([128, 1], F32, tag="a")
                    nc.vector.reciprocal(a[:rows, :], sd[:rows, :])
                    bnn = pl.tile([128, 1], F32, tag="bnn")
                    nc.vector.tensor_scalar(bnn[:rows, :], st2[:rows, 0:1],
                                            a[:rows, :], -1.0, OP.mult, OP.mult)
                    h0 = pl.tile([128, 128], BF16, tag="h0")
                    nc.scalar.activation(h0[:rows, :], xt[:rows, :],
                                         AF.Identity, bias=bnn[:rows, :],
                                         scale=a[:rows, :])
                    ps_t = pps.tile([128, 128], BF16, tag="psT")
                    nc.tensor.matmul(ps_t[:, :rows], h0[:rows, :],
                                     I_bf[:rows, :rows],
                                     is_transpose=True, start=True, stop=True)
                    nc.any.tensor_copy(out=h0T_own[:, nt, :rows],
                                       in_=ps_t[:, :rows])
                    ps_za = pps1.tile([128, 128], F32, tag="psza")
                    nc.tensor.matmul(ps_za[:], lhsT=h0T_own[:, nt, :],
                                     rhs=Wap_s[:], start=True, stop=False,
                                     skip_group_check=True)
                    nc.tensor.matmul(ps_za[:], lhsT=frt[:], rhs=Wfn_s[:],
                                     start=False, stop=True,
                                     skip_group_check=True)
                    nc.any.tensor_copy(out=za_own[:, nt, :], in_=ps_za[:])
                    ps_zb = pps1.tile([128, 128], F32, tag="pszb")
                    nc.tensor.matmul(ps_zb[:], lhsT=h0T_own[:, nt, :],
                                     rhs=Wbp_s[:], start=True, stop=False,
                                     skip_group_check=True)
                    nc.tensor.matmul(ps_zb[:], lhsT=frt[:], rhs=Wfp_s[:],
                                     start=False, stop=True,
                                     skip_group_check=True)
                    zbb = pl.tile([128, 128], BF16, tag="zbb")
                    nc.any.tensor_copy(out=zbb[:], in_=ps_zb[:])
                    nc.sync.dma_start(out=zbslice[nt * 128:nt * 128 + rows, :],
                                      in_=zbb[:rows, :])

            # ---- share zb across cores ----
            nc.gpsimd.collective_compute(
                "AllGather", OP.bypass,
                replica_groups=[list(range(NC))],
                ins=[zbslice[:].opt()],
                outs=[zb_tbl[:].opt()],
            )

            # ---------------- phase 2: edges + node update ----------------
            with (
                tc.tile_pool(name="idx", bufs=2) as pidx,
                tc.tile_pool(name="gat", bufs=2) as pg,
                tc.tile_pool(name="work", bufs=2) as pw,
                tc.tile_pool(name="ps_z1", bufs=2, space="PSUM") as ps_z1,
                tc.tile_pool(name="ps_z2", bufs=2, space="PSUM") as ps_z2,
                tc.tile_pool(name="ps_agg", bufs=2, space="PSUM") as ps_agg,
            ):
                for nt in range(NT):
                    rows = 106 if nt == NT - 1 else 128
                    # ---- index loads ----
                    t_dst = pidx.tile([128, SNT], I32, tag="dst")
                    nc.sync.dma_start(out=t_dst[:], in_=dstT[nt, :, :])
                    t_cid = pidx.tile([128, SNT], I32, tag="cid")
                    nc.sync.dma_start(out=t_cid[:], in_=cidxT[nt, :, :])
                    t_srl16 = pidx.tile([128, SNT], FP16, tag="srl16")
                    nc.sync.dma_start(out=t_srl16[:], in_=srclT[nt, :, :])
                    t_deg16 = pidx.tile([128, SNT], FP16, tag="deg16")
                    nc.sync.dma_start(out=t_deg16[:], in_=degT[nt, :, :])
                    t_str16 = pidx.tile([128, 2 * NCHUNK], FP16, tag="str16")
                    nc.sync.dma_start(out=t_str16[:], in_=stairsT[nt, :, :])

                    t_srl = pidx.tile([128, SNT], F32, tag="srl")
                    nc.any.tensor_copy(out=t_srl[:], in_=t_srl16[:])
                    t_degf = pidx.tile([128, SNT], F32, tag="degf")
                    nc.any.tensor_copy(out=t_degf[:], in_=t_deg16[:])
                    t_inv = pidx.tile([128, SNT], F32, tag="inv")
                    nc.vector.reciprocal(t_inv[:], t_degf[:])
                    t_str = pidx.tile([128, 2 * NCHUNK], F32, tag="str")
                    nc.any.tensor_copy(out=t_str[:], in_=t_str16[:])

                    # ---- gathers (edge-major, one row per partition) ----
                    g_zb = pg.tile([128, SNT, 128], BF16, tag="gzb")
                    g_cb = pg.tile([128, SNT, 128], BF16, tag="gcb")
                    for j in range(SNT):
                        nc.gpsimd.indirect_dma_start(
                            out=g_zb[:, j, :], out_offset=None,
                            in_=zb_tbl[:, :],
                            in_offset=bass.IndirectOffsetOnAxis(
                                ap=t_dst[:, j:j + 1], axis=0))
                        nc.gpsimd.indirect_dma_start(
                            out=g_cb[:, j, :], out_offset=None,
                            in_=comb[:, :],
                            in_offset=bass.IndirectOffsetOnAxis(
                                ap=t_cid[:, j:j + 1], axis=0))

                    agg = ps_agg.tile([128, 128], F32, tag="agg")

                    for ci, (j0, S) in enumerate(CHUNKS):
                        W = S * 128
                        # staircase selection matrix selT [128n, W]
                        t0 = pw.tile([128, 512], BF16, tag="t0")
                        nc.vector.tensor_scalar(
                            t0[:, :W], iota_f[:, :W],
                            t_str[:, 2 * ci + 1:2 * ci + 2], None, OP.is_lt)
                        selT = pw.tile([128, 512], BF16, tag="selT")
                        nc.vector.scalar_tensor_tensor(
                            out=selT[:, :W], in0=iota_f[:, :W],
                            scalar=t_str[:, 2 * ci:2 * ci + 1],
                            in1=t0[:, :W], op0=OP.is_ge, op1=OP.mult)

                        # zb + comb summed, then xbar-transposed to FM
                        gsum = pw.tile([128, 4, 128], BF16, tag="gsum")
                        nc.vector.tensor_tensor(
                            out=gsum[:, :S, :], in0=g_zb[:, j0:j0 + S, :],
                            in1=g_cb[:, j0:j0 + S, :], op=OP.add)
                        gT = pw.tile([128, 4, 128], BF16, tag="gT")
                        nc.sync.dma_start_transpose(gT[:, :S, :],
                                                    gsum[:, :S, :])

                        # z1T accumulation [128H, W]
                        z1 = ps_z1.tile([128, 512], F32, tag="z1")
                        nc.tensor.matmul(z1[:, :W], lhsT=za_own[:, nt, :],
                                         rhs=selT[:, :W], start=True,
                                         stop=False, skip_group_check=True)
                        nc.tensor.matmul(z1[:, :W], lhsT=I_bf[:],
                                         rhs=gT[:, :S, :], start=False,
                                         stop=True, skip_group_check=True)

                        e1T = pw.tile([128, 512], BF16, tag="e1T")
                        nc.scalar.activation(e1T[:, :W], z1[:, :W], AF.Silu)

                        z2 = ps_z2.tile([128, 512], F32, tag="z2")
                        nc.tensor.matmul(z2[:, :W], lhsT=We2_s[:],
                                         rhs=e1T[:, :W], start=True, stop=True)
                        e2T = pw.tile([128, 512], BF16, tag="e2T")
                        nc.scalar.activation(e2T[:, :W], z2[:, :W], AF.Silu,
                                             bias=be2c[:])
                        e2em = pw.tile([128, 4, 128], BF16, tag="e2em")
                        nc.sync.dma_start_transpose(e2em[:, :S, :], e2T[:, :W])

                        # scatter-mean matmuls into agg [128H, 128n]
                        for j in range(S):
                            jj = j0 + j
                            selp = pw.tile([128, 128], BF16, tag="selp")
                            nc.vector.tensor_scalar(
                                selp[:], iota_f[:, :128],
                                t_srl[:, jj:jj + 1], t_inv[:, jj:jj + 1],
                                OP.is_equal, OP.mult)
                            nc.tensor.matmul(
                                agg[:], lhsT=e2em[:, j, :], rhs=selp[:],
                                start=(ci == 0 and j == 0),
                                stop=(ci == NCHUNK - 1 and j == S - 1),
                                skip_group_check=True)

                    # ---- node update for this tile ----
                    aggb = pw.tile([128, 128], BF16, tag="aggb")
                    nc.any.tensor_copy(out=aggb[:], in_=agg[:])
                    n1 = ps_z1.tile([128, 512], F32, tag="z1")
                    nc.tensor.matmul(n1[:, :128], lhsT=Wn1h_s[:],
                                     rhs=h0T_own[:, nt, :], start=True,
                                     stop=False, skip_group_check=True)
                    nc.tensor.matmul(n1[:, :128], lhsT=Wn1a_s[:], rhs=aggb[:],
                                     start=False, stop=True,
                                     skip_group_check=True)
                    n1T = pw.tile([128, 128], BF16, tag="n1T")
                    nc.scalar.activation(n1T[:], n1[:, :128], AF.Silu,
                                         bias=bn1c[:])
                    n2 = ps_z2.tile([128, 512], F32, tag="z2")
                    nc.tensor.matmul(n2[:, :128], lhsT=Wn2_s[:], rhs=n1T[:],
                                     start=True, stop=True)
                    n2T = pw.tile([128, 128], BF16, tag="n2T")
                    nc.scalar.activation(n2T[:], n2[:, :128], AF.Silu,
                                         bias=bn2c[:])
                    n2em = pw.tile([128, 1, 128], BF16, tag="n2em")
                    nc.sync.dma_start_transpose(n2em[:], n2T[:])
                    nc.sync.dma_start(out=out[nt * 128:nt * 128 + rows, :],
                                      in_=n2em[:rows, 0, :])
    nc.finalize()
    return nc


# --------------------------------------------------------------------------
# cached jit runner (trace/lower/compile once per process)
# --------------------------------------------------------------------------

class _Result:
    exec_time_ns = None
    profile_json = None
    mean_exec_time_ns = None
    results = None


class _Runner:
    def __init__(self):
        import jax
        import jax.numpy as jnp
        from jax.sharding import Mesh, PartitionSpec, NamedSharding
        from jax.experimental.shard_map import shard_map
        from concourse.bass2jax import (
            _bass_exec_p, install_neuronx_cc_hook, partition_id_tensor)

        self.jax = jax
        nc = build_program()
        self.nc = nc
        install_neuronx_cc_hook()

        partition_name = (nc.partition_id_tensor.name
                          if nc.partition_id_tensor else None)
        in_names, out_names, out_avals = [], [], []
        for alloc in nc.m.functions[0].allocations:
            if not isinstance(alloc, mybir.MemoryLocationSet):
                continue
            name = alloc.memorylocations[0].name
            if alloc.kind == "ExternalInput":
                if name != partition_name:
                    in_names.append(name)
            elif alloc.kind == "ExternalOutput":
                out_names.append(name)
                out_avals.append(jax.core.ShapedArray(
                    tuple(alloc.tensor_shape), mybir.dt.np(alloc.dtype)))
        self.in_names, self.out_names = in_names, out_names
        n_params, n_outs = len(in_names), len(out_avals)
        all_in = tuple(in_names + out_names
                       + ([partition_name] if partition_name else []))

        def _body(*args):
            operands = list(args)
            if partition_name is not None:
                operands.append(partition_id_tensor())
            outs = _bass_exec_p.bind(
                *operands, out_avals=tuple(out_avals), in_names=all_in,
                out_names=tuple(out_names), lowering_input_output_aliases=(),
                sim_require_finite=True, sim_require_nnan=True, nc=nc)
            return tuple(outs)

        devices = jax.devices()[:NC]
        assert len(devices) == NC
        mesh = Mesh(np.asarray(devices), ("core",))
        PS = PartitionSpec
        donate = tuple(range(n_params, n_params + n_outs))
        self.fn = jax.jit(
            shard_map(_body, mesh=mesh,
                      in_specs=(PS("core"),) * (n_params + n_outs),
                      out_specs=(PS("core"),) * n_outs, check_rep=False),
            donate_argnums=donate, keep_unused=True)

        sh = NamedSharding(mesh, PS("core"))
        zshapes = [(NC * a.shape[0], *a.shape[1:]) for a in out_avals]
        zdtypes = [a.dtype for a in out_avals]
        self.make_zeros = jax.jit(
            lambda: tuple(jnp.zeros(s, d) for s, d in zip(zshapes, zdtypes)),
            out_shardings=(sh,) * n_outs)

    def __call__(self, arg_map):
        args = [arg_map[n] for n in self.in_names]
        zeros = self.make_zeros()
        outs = self.fn(*args, *zeros)
        return {name: outs[i] for i, name in enumerate(self.out_names)}

    @staticmethod
    def fetch(arr):
        # per-shard parallel device->host pull: ~70x faster than a
        # sequential np.asarray on the global array under axon
        from concurrent.futures import ThreadPoolExecutor
        shards = arr.addressable_shards
        with ThreadPoolExecutor(len(shards)) as ex:
            parts = list(ex.map(lambda s: np.asarray(s.data), shards))
        return np.concatenate(parts, axis=0)


_RUNNER = None


def kernel(**inputs) -> np.ndarray:
    out, _ = run(inputs, trace=False)
    return out


def run(inputs, trace=False):
    global _RUNNER
    if _RUNNER is None:
        _RUNNER = _Runner()

    x = np.ascontiguousarray(np.asarray(inputs["node_features"], np.float32))
    idx = _host_prep(inputs)
    wts = _host_weights(inputs)

    am = dict(xown=x.astype(BF16NP))
    am.update(idx)
    for k, v in wts.items():
        # replicate per core along axis 0
        am[k] = np.tile(v, (NC,) + (1,) * (v.ndim - 1)) if v.ndim > 1 \
            else np.tile(v, NC)

    outs = _RUNNER(am)
    nout = _Runner.fetch(outs["nout"])  # [N, H] bf16, core-order == node-order
    res = _Result()
    return x + nout.astype(np.float32), res


if __name__ == "__main__":
    build_program()
    print("program built OK")


# revision 13
# speedup vs baseline: 9.4380x; 1.3066x over previous
"""Trainium2 Bass kernel for nn_CSPLayer (GNN message passing), 8 NeuronCores.

Strategy: sort edges by src node; core c owns nodes [c*6250,(c+1)*6250) and all
their outgoing edges (scatter over src is then core-local). Per core the edges
are grouped by 128-node tiles, each padded to a fixed 2304 slots so every core
runs an identical instruction stream (SPMD).

v2 pipeline changes vs v1:
  - x is sharded (each core receives only its own 6250 rows); every core
    computes the zb table for its own nodes and an on-device AllGather
    builds the full [N,H] zb table each core gathers from.  This removes
    the 8x-replicated 25.6MB x input (205MB -> 25.6MB host->device).
  - frac terms are folded into per-node tables: frac_diff = fj - fi + k
    with k in {0,1}^3 the mod-1 wrap bits (computed exactly on host).
    fj@Wf folds into zb, -fi@Wf into za, and k@Wf + lat_ip@Wl + be1tot
    into a 1024-row combined table indexed by (k*128 + graph).  This
    eliminates the frac gathers and all per-edge frac math.
  - gamma/beta/biases folded into bf16 weights on the host.
  - jit runner is cached module-wide (trace/lower/compile once); donated
    output zero-buffers are created on device; output is n (bf16), the
    residual x + n is added on the host in f32.

Math:
  h   = LN(x);  h0 = (x-mu)*rsqrt(var+eps)   (gamma/beta folded into weights)
  za  = h0 @ (gamma*Wa) - frac @ Wf          (own nodes, SBUF resident, bf16)
  zb  = h0 @ (gamma*Wb) + frac @ Wf          (own slice -> AllGather -> [N,H])
  comb[k*128+g] = (L L^T)[g] @ Wl + be1 + beta@(Wa+Wb) + k @ Wf
  z1T[:,e] = za[src] (stair-matmul) + zb[dst]^T + comb[kcode,e2g]^T
  e1 = silu(z1); e2 = silu(e1@We2+be2); agg = scatter-mean over src
  n  = silu(silu([h|agg]@Wn1+bn1)@Wn2+bn2);  out = x + n (host add)
"""

import os
import sys

import numpy as np

if "/opt/trn_rl_repo" not in sys.path:
    sys.path.insert(0, "/opt/trn_rl_repo")

import concourse.bass as bass
import concourse.bacc as bacc
import concourse.mybir as mybir
import concourse.tile as tile
from concourse.masks import make_identity

import ml_dtypes

BF16NP = ml_dtypes.bfloat16

F32 = mybir.dt.float32
BF16 = mybir.dt.bfloat16
FP16 = mybir.dt.float16
I32 = mybir.dt.int32

N, E, G, H = 50000, 800000, 128, 128
NC = 8
NPC = N // NC            # 6250 nodes per core
NT = 49                  # node tiles per core (48*128 + 106)
ENT = 2304               # padded edge slots per node tile (18 subchunks)
SNT = ENT // 128         # 18 subchunks of 128 edges
# FM chunk plan: (subchunk offset j0, subchunk count S)
CHUNKS = [(0, 4), (4, 4), (8, 4), (12, 4), (16, 2)]
NCHUNK = len(CHUNKS)
EPS = 1e-5
AF = mybir.ActivationFunctionType
OP = mybir.AluOpType


# --------------------------------------------------------------------------
# host-side prep: pure index manipulation / padding / layout
# --------------------------------------------------------------------------

def _host_prep(inputs):
    src = np.asarray(inputs["edge_index"][0]).astype(np.int32)
    dst = np.asarray(inputs["edge_index"][1]).astype(np.int32)
    e2g = np.asarray(inputs["edge2graph"]).astype(np.int32)
    fr = np.asarray(inputs["frac_coords"], np.float32)
    deg = np.bincount(src, minlength=N)
    perm = np.argsort(src, kind="stable")
    srcS, dstS, e2gS = src[perm], dst[perm], e2g[perm]

    # mod-1 wrap bits per edge (exact, from f32 coords)
    d3 = fr[dstS] - fr[srcS]
    kcode = ((d3[:, 0] < 0).astype(np.int32)
             + 2 * (d3[:, 1] < 0).astype(np.int32)
             + 4 * (d3[:, 2] < 0).astype(np.int32))
    cidxS = kcode * G + e2gS

    # edge -> (tile row, slot) fully vectorized: edges sorted by src are
    # contiguous per 128-node tile; slot = edge rank within its tile.
    cum = np.zeros(N + 1, np.int64)
    np.cumsum(deg, out=cum[1:])
    c_of = srcS // NPC
    loc = srcS - c_of * NPC
    nt_of = loc >> 7
    row_of = c_of * NT + nt_of
    first_node = c_of * NPC + (nt_of << 7)
    slot = np.arange(E, dtype=np.int64) - cum[first_node]
    assert slot.max() < ENT, f"node tile overflow: {slot.max() + 1} > {ENT}"
    part = (slot & 127).astype(np.int64)
    col = (slot >> 7).astype(np.int64)

    # dst in low 16 bits, comb index in high 16 (device unpacks via and/shr)
    pidxT = np.zeros((NC * NT, 128, SNT), np.int32)
    pidxT[row_of, part, col] = (cidxS << 16) | dstS

    # per-node degree column (stairs/selT/selp/inv-deg derive on device)
    valid = np.arange(NT * 128) < NPC
    degN = np.zeros((NC, NT * 128), np.float16)
    degN[:, valid] = deg.reshape(NC, NPC).astype(np.float16)
    degN = degN.reshape(NC * NT, 128, 1)

    # frac transposed per own-node tile: frT[c*NT+nt, comp, p] = fr[node, comp]
    frT = np.zeros((NC, NT * 128, 4), np.float32)
    frT[:, valid, :3] = fr.reshape(NC, NPC, 3)
    frT = np.ascontiguousarray(
        frT.reshape(NC * NT, 128, 4).transpose(0, 2, 1)).astype(BF16NP)

    return dict(pidxT=pidxT, degN=degN, frT=frT)


def _host_weights(inputs):
    gam = np.asarray(inputs["gamma"], np.float32)
    bet = np.asarray(inputs["beta"], np.float32)
    We1 = np.asarray(inputs["We1"], np.float32)
    Wa, Wb = We1[0:128], We1[128:256]
    Wl, Wf = We1[256:265], We1[265:268]
    be1tot = np.asarray(inputs["be1"], np.float32) + bet @ (Wa + Wb)

    lat = np.asarray(inputs["lattices"], np.float32)
    lat_ip = np.einsum("gij,gkj->gik", lat, lat).reshape(G, 9)
    wlat = lat_ip @ Wl + be1tot  # [G, H]
    kmat = np.array([[(b >> c) & 1 for c in range(3)] for b in range(8)],
                    np.float32)
    kWf = kmat @ Wf  # [8, H]
    comb = (wlat[None, :, :] + kWf[:, None, :]).reshape(8 * G, H)

    def pad4(w):
        out = np.zeros((4, H), np.float32)
        out[:3] = w
        return out

    Wn1 = np.asarray(inputs["Wn1"], np.float32)
    Wn1h, Wn1a = Wn1[0:128], Wn1[128:256]
    bn1tot = np.asarray(inputs["bn1"], np.float32) + bet @ Wn1h

    return dict(
        Wap=(gam[:, None] * Wa).astype(BF16NP),
        Wbp=(gam[:, None] * Wb).astype(BF16NP),
        Wfp=pad4(Wf).astype(BF16NP),
        Wfn=pad4(-Wf).astype(BF16NP),
        comb=comb.astype(BF16NP),
        We2b=np.asarray(inputs["We2"], np.float32).astype(BF16NP),
        be2=np.asarray(inputs["be2"], np.float32),
        Wn1hb=(gam[:, None] * Wn1h).astype(BF16NP),
        Wn1ab=Wn1a.astype(BF16NP),
        bn1t=bn1tot,
        Wn2b=np.asarray(inputs["Wn2"], np.float32).astype(BF16NP),
        bn2=np.asarray(inputs["bn2"], np.float32),
    )


# --------------------------------------------------------------------------
# bass program (single SPMD program for all 8 cores)
# --------------------------------------------------------------------------

def build_program():
    nc = bacc.Bacc()
    p = lambda n, s, d: nc.declare_dram_parameter(n, list(s), d, isOutput=False)

    xown = p("xown", (NPC, H), BF16)
    frT = p("frT", (NT, 4, 128), BF16)
    pidxT = p("pidxT", (NT, 128, SNT), I32)
    degN = p("degN", (NT, 128, 1), FP16)
    comb = p("comb", (8 * G, H), BF16)
    Wap = p("Wap", (H, H), BF16)
    Wbp = p("Wbp", (H, H), BF16)
    Wfp = p("Wfp", (4, H), BF16)
    Wfn = p("Wfn", (4, H), BF16)
    We2b = p("We2b", (H, H), BF16)
    Wn1hb = p("Wn1hb", (H, H), BF16)
    Wn1ab = p("Wn1ab", (H, H), BF16)
    Wn2b = p("Wn2b", (H, H), BF16)
    be2 = p("be2", (H,), F32)
    bn1t = p("bn1t", (H,), F32)
    bn2 = p("bn2", (H,), F32)

    out = nc.declare_dram_parameter("nout", [NPC, H], BF16, isOutput=True)

    with tile.TileContext(nc) as tc:
        with (
            tc.tile_pool(name="dram", bufs=1, space="DRAM") as dram,
            tc.tile_pool(name="persist", bufs=1) as pp,
        ):
            zbslice = dram.tile([NPC, H], BF16)
            zb_tbl = dram.tile([N, H], BF16)

            # ---------------- constants ----------------
            I_bf = pp.tile([128, 128], BF16)
            make_identity(nc, I_bf[:])
            iota_i = pp.tile([128, 512], I32)
            nc.gpsimd.iota(iota_i[:], pattern=[[1, 512]], base=0,
                           channel_multiplier=0)
            iota_f = pp.tile([128, 512], F32)
            nc.any.tensor_copy(out=iota_f[:], in_=iota_i[:])
            # partition-index column and strict-upper-triangular ones matrix
            # (UT[q,p] = 1 iff q < p) for on-device prefix sums of degrees
            iotac_i = pp.tile([128, 1], I32)
            nc.gpsimd.iota(iotac_i[:], pattern=[[1, 1]], base=0,
                           channel_multiplier=1)
            iotac_f = pp.tile([128, 1], F32)
            nc.any.tensor_copy(out=iotac_f[:], in_=iotac_i[:])
            UT_bf = pp.tile([128, 128], BF16)
            nc.vector.tensor_scalar(UT_bf[:], iota_f[:, :128], iotac_f[:],
                                    None, OP.is_gt)

            def load_col(ap, tag):
                t = pp.tile([128, 1], F32, tag=tag)
                nc.sync.dma_start(out=t[:], in_=ap[:, None])
                return t

            be2c = load_col(be2, "be2c")
            bn1c = load_col(bn1t, "bn1c")
            bn2c = load_col(bn2, "bn2c")
            epsc = pp.tile([128, 1], F32)
            nc.gpsimd.memset(epsc[:], EPS)

            def load_w(ap, shape, tag):
                t = pp.tile(list(shape), BF16, tag=tag)
                nc.sync.dma_start(out=t[:], in_=ap[:, :])
                return t

            Wap_s = load_w(Wap, (128, 128), "Wap_s")
            Wbp_s = load_w(Wbp, (128, 128), "Wbp_s")
            Wfp_s = load_w(Wfp, (4, 128), "Wfp_s")
            Wfn_s = load_w(Wfn, (4, 128), "Wfn_s")
            We2_s = load_w(We2b, (128, 128), "We2_s")
            Wn1h_s = load_w(Wn1hb, (128, 128), "Wn1h_s")
            Wn1a_s = load_w(Wn1ab, (128, 128), "Wn1a_s")
            Wn2_s = load_w(Wn2b, (128, 128), "Wn2_s")

            # persistent per-core state
            za_own = pp.tile([128, NT, 128], BF16)
            h0T_own = pp.tile([128, NT, 128], BF16)
            nc.gpsimd.memset(za_own[:], 0.0)
            nc.gpsimd.memset(h0T_own[:], 0.0)

            # ---- phase 1: own nodes -> h0T_own, za_own, zbslice ----
            with (
                tc.tile_pool(name="p1", bufs=3) as pl,
                tc.tile_pool(name="p1psT", bufs=2, space="PSUM") as pps,
                tc.tile_pool(name="p1psZ", bufs=2, space="PSUM") as pps1,
            ):
                for nt in range(NT):
                    rows = 106 if nt == NT - 1 else 128
                    xt_b = pl.tile([128, 128], BF16, tag="xtb")
                    nc.sync.dma_start(out=xt_b[:rows, :],
                                      in_=xown[nt * 128:nt * 128 + rows, :])
                    xt = pl.tile([128, 128], F32, tag="xt")
                    nc.any.tensor_copy(out=xt[:rows, :], in_=xt_b[:rows, :])
                    frt = pl.tile([4, 128], BF16, tag="frt")
                    nc.sync.dma_start(out=frt[:], in_=frT[nt, :, :])
                    st6 = pl.tile([128, 6], F32, tag="st6")
                    nc.vector.bn_stats(st6[:rows, :], xt[:rows, :])
                    st2 = pl.tile([128, 2], F32, tag="st2")
                    nc.vector.bn_aggr(st2[:rows, :], st6[:rows, :])
                    sd = pl.tile([128, 1], F32, tag="sd")
                    nc.scalar.activation(sd[:rows, :], st2[:rows, 1:2],
                                         AF.Sqrt, bias=epsc[:rows, :])
                    a = pl.tile([128, 1], F32, tag="a")
                    nc.vector.reciprocal(a[:rows, :], sd[:rows, :])
                    bnn = pl.tile([128, 1], F32, tag="bnn")
                    nc.vector.tensor_scalar(bnn[:rows, :], st2[:rows, 0:1],
                                            a[:rows, :], -1.0, OP.mult, OP.mult)
                    h0 = pl.tile([128, 128], BF16, tag="h0")
                    nc.scalar.activation(h0[:rows, :], xt[:rows, :],
                                         AF.Identity, bias=bnn[:rows, :],
                                         scale=a[:rows, :])
                    ps_t = pps.tile([128, 128], BF16, tag="psT")
                    nc.tensor.matmul(ps_t[:, :rows], h0[:rows, :],
                                     I_bf[:rows, :rows],
                                     is_transpose=True, start=True, stop=True)
                    nc.any.tensor_copy(out=h0T_own[:, nt, :rows],
                                       in_=ps_t[:, :rows])
                    ps_za = pps1.tile([128, 128], F32, tag="psza")
                    nc.tensor.matmul(ps_za[:], lhsT=h0T_own[:, nt, :],
                                     rhs=Wap_s[:], start=True, stop=False,
                                     skip_group_check=True)
                    nc.tensor.matmul(ps_za[:], lhsT=frt[:], rhs=Wfn_s[:],
                                     start=False, stop=True,
                                     skip_group_check=True)
                    nc.any.tensor_copy(out=za_own[:, nt, :], in_=ps_za[:])
                    ps_zb = pps1.tile([128, 128], F32, tag="pszb")
                    nc.tensor.matmul(ps_zb[:], lhsT=h0T_own[:, nt, :],
                                     rhs=Wbp_s[:], start=True, stop=False,
                                     skip_group_check=True)
                    nc.tensor.matmul(ps_zb[:], lhsT=frt[:], rhs=Wfp_s[:],
                                     start=False, stop=True,
                                     skip_group_check=True)
                    zbb = pl.tile([128, 128], BF16, tag="zbb")
                    nc.any.tensor_copy(out=zbb[:], in_=ps_zb[:])
                    nc.sync.dma_start(out=zbslice[nt * 128:nt * 128 + rows, :],
                                      in_=zbb[:rows, :])

            # ---- share zb across cores ----
            nc.gpsimd.collective_compute(
                "AllGather", OP.bypass,
                replica_groups=[list(range(NC))],
                ins=[zbslice[:].opt()],
                outs=[zb_tbl[:].opt()],
            )

            # ---------------- phase 2: edges + node update ----------------
            with (
                tc.tile_pool(name="idx", bufs=2) as pidx,
                tc.tile_pool(name="gat", bufs=2) as pg,
                tc.tile_pool(name="work", bufs=2) as pw,
                tc.tile_pool(name="ps_z1", bufs=2, space="PSUM") as ps_z1,
                tc.tile_pool(name="ps_z2", bufs=2, space="PSUM") as ps_z2,
                tc.tile_pool(name="ps_agg", bufs=2, space="PSUM") as ps_agg,
                tc.tile_pool(name="ps_sm", bufs=1, space="PSUM") as ps_sm,
            ):
                for nt in range(NT):
                    rows = 106 if nt == NT - 1 else 128
                    # ---- index loads + unpack ----
                    t_pid = pidx.tile([128, SNT], I32, tag="pid")
                    nc.sync.dma_start(out=t_pid[:], in_=pidxT[nt, :, :])
                    t_dst = pidx.tile([128, SNT], I32, tag="dst")
                    nc.vector.tensor_scalar(t_dst[:], t_pid[:], 65535, None,
                                            OP.bitwise_and)
                    t_cid = pidx.tile([128, SNT], I32, tag="cid")
                    nc.vector.tensor_scalar(t_cid[:], t_pid[:], 16, None,
                                            OP.logical_shift_right)

                    # ---- per-node degree -> stair bounds + 1/deg ----
                    dcol16 = pidx.tile([128, 1], FP16, tag="dc16")
                    nc.sync.dma_start(out=dcol16[:], in_=degN[nt, :, :])
                    dcol = pidx.tile([128, 1], F32, tag="dcol")
                    nc.any.tensor_copy(out=dcol[:], in_=dcol16[:])
                    dcol_bf = pidx.tile([128, 1], BF16, tag="dcbf")
                    nc.any.tensor_copy(out=dcol_bf[:], in_=dcol16[:])
                    dmax = pidx.tile([128, 1], F32, tag="dmax")
                    nc.vector.tensor_scalar(dmax[:], dcol[:], 1.0, None,
                                            OP.max)
                    t_invn = pidx.tile([128, 1], F32, tag="invn")
                    nc.vector.reciprocal(t_invn[:], dmax[:])
                    ps_st = ps_sm.tile([128, 1], F32, tag="psst")
                    nc.tensor.matmul(ps_st[:], lhsT=UT_bf[:], rhs=dcol_bf[:],
                                     start=True, stop=True)
                    st_col = pidx.tile([128, 1], F32, tag="stc")
                    nc.any.tensor_copy(out=st_col[:], in_=ps_st[:])
                    en_col = pidx.tile([128, 1], F32, tag="enc")
                    nc.vector.tensor_tensor(out=en_col[:], in0=st_col[:],
                                            in1=dcol[:], op=OP.add)

                    # ---- gathers (edge-major, one row per partition) ----
                    g_zb = pg.tile([128, SNT, 128], BF16, tag="gzb")
                    g_cb = pg.tile([128, SNT, 128], BF16, tag="gcb")
                    for j in range(SNT):
                        nc.gpsimd.indirect_dma_start(
                            out=g_zb[:, j, :], out_offset=None,
                            in_=zb_tbl[:, :],
                            in_offset=bass.IndirectOffsetOnAxis(
                                ap=t_dst[:, j:j + 1], axis=0))
                        nc.gpsimd.indirect_dma_start(
                            out=g_cb[:, j, :], out_offset=None,
                            in_=comb[:, :],
                            in_offset=bass.IndirectOffsetOnAxis(
                                ap=t_cid[:, j:j + 1], axis=0))

                    agg = ps_agg.tile([128, 128], F32, tag="agg")

                    for ci, (j0, S) in enumerate(CHUNKS):
                        W = S * 128
                        base = float(j0 * 128)
                        stb = pw.tile([128, 1], F32, tag="stb")
                        nc.vector.tensor_scalar(stb[:], st_col[:], base, None,
                                                OP.subtract)
                        enb = pw.tile([128, 1], F32, tag="enb")
                        nc.vector.tensor_scalar(enb[:], en_col[:], base, None,
                                                OP.subtract)
                        # staircase selection matrix selT [128n, W]
                        t0 = pw.tile([128, 512], BF16, tag="t0")
                        nc.vector.tensor_scalar(
                            t0[:, :W], iota_f[:, :W], enb[:], None, OP.is_lt)
                        selT = pw.tile([128, 512], BF16, tag="selT")
                        nc.vector.scalar_tensor_tensor(
                            out=selT[:, :W], in0=iota_f[:, :W],
                            scalar=stb[:], in1=t0[:, :W],
                            op0=OP.is_ge, op1=OP.mult)
                        # selT with 1/deg folded per node row (scatter-mean)
                        selTs = pw.tile([128, 512], BF16, tag="selTs")
                        nc.scalar.activation(selTs[:, :W], selT[:, :W],
                                             AF.Identity, scale=t_invn[:])

                        # zb + comb summed, then xbar-transposed to FM
                        gsum = pw.tile([128, 4, 128], BF16, tag="gsum")
                        nc.vector.tensor_tensor(
                            out=gsum[:, :S, :], in0=g_zb[:, j0:j0 + S, :],
                            in1=g_cb[:, j0:j0 + S, :], op=OP.add)
                        gT = pw.tile([128, 4, 128], BF16, tag="gT")
                        nc.sync.dma_start_transpose(gT[:, :S, :],
                                                    gsum[:, :S, :])

                        # z1T accumulation [128H, W]
                        z1 = ps_z1.tile([128, 512], F32, tag="z1")
                        nc.tensor.matmul(z1[:, :W], lhsT=za_own[:, nt, :],
                                         rhs=selT[:, :W], start=True,
                                         stop=False, skip_group_check=True)
                        nc.tensor.matmul(z1[:, :W], lhsT=I_bf[:],
                                         rhs=gT[:, :S, :], start=False,
                                         stop=True, skip_group_check=True)

                        e1T = pw.tile([128, 512], BF16, tag="e1T")
                        nc.scalar.activation(e1T[:, :W], z1[:, :W], AF.Silu)

                        z2 = ps_z2.tile([128, 512], F32, tag="z2")
                        nc.tensor.matmul(z2[:, :W], lhsT=We2_s[:],
                                         rhs=e1T[:, :W], start=True, stop=True)
                        e2T = pw.tile([128, 512], BF16, tag="e2T")
                        nc.scalar.activation(e2T[:, :W], z2[:, :W], AF.Silu,
                                             bias=be2c[:])
                        e2em = pw.tile([128, 4, 128], BF16, tag="e2em")
                        nc.sync.dma_start_transpose(e2em[:, :S, :], e2T[:, :W])

                        # scatter-mean matmuls into agg [128H, 128n]:
                        # selp = (selTs subchunk)^T via PE transpose
                        for j in range(S):
                            ps_sp = ps_sm.tile([128, 128], BF16, tag="pssp")
                            nc.tensor.matmul(
                                ps_sp[:], selTs[:, j * 128:(j + 1) * 128],
                                I_bf[:], is_transpose=True,
                                start=True, stop=True)
                            selp = pw.tile([128, 128], BF16, tag="selp")
                            nc.any.tensor_copy(out=selp[:], in_=ps_sp[:])
                            nc.tensor.matmul(
                                agg[:], lhsT=e2em[:, j, :], rhs=selp[:],
                                start=(ci == 0 and j == 0),
                                stop=(ci == NCHUNK - 1 and j == S - 1),
                                skip_group_check=True)

                    # ---- node update for this tile ----
                    aggb = pw.tile([128, 128], BF16, tag="aggb")
                    nc.any.tensor_copy(out=aggb[:], in_=agg[:])
                    n1 = ps_z1.tile([128, 512], F32, tag="z1")
                    nc.tensor.matmul(n1[:, :128], lhsT=Wn1h_s[:],
                                     rhs=h0T_own[:, nt, :], start=True,
                                     stop=False, skip_group_check=True)
                    nc.tensor.matmul(n1[:, :128], lhsT=Wn1a_s[:], rhs=aggb[:],
                                     start=False, stop=True,
                                     skip_group_check=True)
                    n1T = pw.tile([128, 128], BF16, tag="n1T")
                    nc.scalar.activation(n1T[:], n1[:, :128], AF.Silu,
                                         bias=bn1c[:])
                    n2 = ps_z2.tile([128, 512], F32, tag="z2")
                    nc.tensor.matmul(n2[:, :128], lhsT=Wn2_s[:], rhs=n1T[:],
                                     start=True, stop=True)
                    n2T = pw.tile([128, 128], BF16, tag="n2T")
                    nc.scalar.activation(n2T[:], n2[:, :128], AF.Silu,
                                         bias=bn2c[:])
                    n2em = pw.tile([128, 1, 128], BF16, tag="n2em")
                    nc.sync.dma_start_transpose(n2em[:], n2T[:])
                    nc.sync.dma_start(out=out[nt * 128:nt * 128 + rows, :],
                                      in_=n2em[:rows, 0, :])
    nc.finalize()
    return nc


# --------------------------------------------------------------------------
# cached jit runner (trace/lower/compile once per process)
# --------------------------------------------------------------------------

class _Result:
    exec_time_ns = None
    profile_json = None
    mean_exec_time_ns = None
    results = None


class _Runner:
    def __init__(self):
        import jax
        import jax.numpy as jnp
        from jax.sharding import Mesh, PartitionSpec, NamedSharding
        from jax.experimental.shard_map import shard_map
        from concourse.bass2jax import (
            _bass_exec_p, install_neuronx_cc_hook, partition_id_tensor)

        self.jax = jax
        nc = build_program()
        self.nc = nc
        install_neuronx_cc_hook()

        partition_name = (nc.partition_id_tensor.name
                          if nc.partition_id_tensor else None)
        in_names, out_names, out_avals = [], [], []
        for alloc in nc.m.functions[0].allocations:
            if not isinstance(alloc, mybir.MemoryLocationSet):
                continue
            name = alloc.memorylocations[0].name
            if alloc.kind == "ExternalInput":
                if name != partition_name:
                    in_names.append(name)
            elif alloc.kind == "ExternalOutput":
                out_names.append(name)
                out_avals.append(jax.core.ShapedArray(
                    tuple(alloc.tensor_shape), mybir.dt.np(alloc.dtype)))
        self.in_names, self.out_names = in_names, out_names
        n_params, n_outs = len(in_names), len(out_avals)
        all_in = tuple(in_names + out_names
                       + ([partition_name] if partition_name else []))

        def _body(*args):
            operands = list(args)
            if partition_name is not None:
                operands.append(partition_id_tensor())
            outs = _bass_exec_p.bind(
                *operands, out_avals=tuple(out_avals), in_names=all_in,
                out_names=tuple(out_names), lowering_input_output_aliases=(),
                sim_require_finite=True, sim_require_nnan=True, nc=nc)
            return tuple(outs)

        devices = jax.devices()[:NC]
        assert len(devices) == NC
        mesh = Mesh(np.asarray(devices), ("core",))
        PS = PartitionSpec
        donate = tuple(range(n_params, n_params + n_outs))
        self.fn = jax.jit(
            shard_map(_body, mesh=mesh,
                      in_specs=(PS("core"),) * (n_params + n_outs),
                      out_specs=(PS("core"),) * n_outs, check_rep=False),
            donate_argnums=donate, keep_unused=True)

        sh = NamedSharding(mesh, PS("core"))
        zshapes = [(NC * a.shape[0], *a.shape[1:]) for a in out_avals]
        zdtypes = [a.dtype for a in out_avals]
        self.make_zeros = jax.jit(
            lambda: tuple(jnp.zeros(s, d) for s, d in zip(zshapes, zdtypes)),
            out_shardings=(sh,) * n_outs)

    def __call__(self, arg_map):
        args = [arg_map[n] for n in self.in_names]
        zeros = self.make_zeros()
        outs = self.fn(*args, *zeros)
        return {name: outs[i] for i, name in enumerate(self.out_names)}

    @staticmethod
    def fetch(arr):
        # per-shard parallel device->host pull: ~70x faster than a
        # sequential np.asarray on the global array under axon
        from concurrent.futures import ThreadPoolExecutor
        shards = arr.addressable_shards
        with ThreadPoolExecutor(len(shards)) as ex:
            parts = list(ex.map(lambda s: np.asarray(s.data), shards))
        return np.concatenate(parts, axis=0)


_RUNNER = None


def kernel(**inputs) -> np.ndarray:
    out, _ = run(inputs, trace=False)
    return out


def run(inputs, trace=False):
    global _RUNNER
    if _RUNNER is None:
        _RUNNER = _Runner()

    x = np.ascontiguousarray(np.asarray(inputs["node_features"], np.float32))
    idx = _host_prep(inputs)
    wts = _host_weights(inputs)

    am = dict(xown=x.astype(BF16NP))
    am.update(idx)
    for k, v in wts.items():
        # replicate per core along axis 0
        am[k] = np.tile(v, (NC,) + (1,) * (v.ndim - 1)) if v.ndim > 1 \
            else np.tile(v, NC)

    outs = _RUNNER(am)
    nout = _Runner.fetch(outs["nout"])  # [N, H] bf16, core-order == node-order
    res = _Result()
    return x + nout.astype(np.float32), res


if __name__ == "__main__":
    build_program()
    print("program built OK")


# revision 16
# speedup vs baseline: 10.7612x; 1.1402x over previous
"""Trainium2 Bass kernel for nn_CSPLayer (GNN message passing), 8 NeuronCores.

Strategy: sort edges by src node; core c owns nodes [c*6250,(c+1)*6250) and all
their outgoing edges (scatter over src is then core-local). Per core the edges
are grouped by 128-node tiles, each padded to a fixed 2304 slots so every core
runs an identical instruction stream (SPMD).

v2 pipeline changes vs v1:
  - x is sharded (each core receives only its own 6250 rows); every core
    computes the zb table for its own nodes and an on-device AllGather
    builds the full [N,H] zb table each core gathers from.  This removes
    the 8x-replicated 25.6MB x input (205MB -> 25.6MB host->device).
  - frac terms are folded into per-node tables: frac_diff = fj - fi + k
    with k in {0,1}^3 the mod-1 wrap bits (computed exactly on host).
    fj@Wf folds into zb, -fi@Wf into za, and k@Wf + lat_ip@Wl + be1tot
    into a 1024-row combined table indexed by (k*128 + graph).  This
    eliminates the frac gathers and all per-edge frac math.
  - gamma/beta/biases folded into bf16 weights on the host.
  - jit runner is cached module-wide (trace/lower/compile once); donated
    output zero-buffers are created on device; output is n (bf16), the
    residual x + n is added on the host in f32.

Math:
  h   = LN(x);  h0 = (x-mu)*rsqrt(var+eps)   (gamma/beta folded into weights)
  za  = h0 @ (gamma*Wa) - frac @ Wf          (own nodes, SBUF resident, bf16)
  zb  = h0 @ (gamma*Wb) + frac @ Wf          (own slice -> AllGather -> [N,H])
  comb[k*128+g] = (L L^T)[g] @ Wl + be1 + beta@(Wa+Wb) + k @ Wf
  z1T[:,e] = za[src] (stair-matmul) + zb[dst]^T + comb[kcode,e2g]^T
  e1 = silu(z1); e2 = silu(e1@We2+be2); agg = scatter-mean over src
  n  = silu(silu([h|agg]@Wn1+bn1)@Wn2+bn2);  out = x + n (host add)
"""

import os
import sys

import numpy as np

if "/opt/trn_rl_repo" not in sys.path:
    sys.path.insert(0, "/opt/trn_rl_repo")

import concourse.bass as bass
import concourse.bacc as bacc
import concourse.mybir as mybir
import concourse.tile as tile
from concourse.masks import make_identity

import ml_dtypes

BF16NP = ml_dtypes.bfloat16

F32 = mybir.dt.float32
BF16 = mybir.dt.bfloat16
FP16 = mybir.dt.float16
I32 = mybir.dt.int32

N, E, G, H = 50000, 800000, 128, 128
NC = 8
NPC = N // NC            # 6250 nodes per core
NT = 49                  # node tiles per core (48*128 + 106)
ENT = 2304               # padded edge slots per node tile (18 subchunks)
SNT = ENT // 128         # 18 subchunks of 128 edges
# FM chunk plan: (subchunk offset j0, subchunk count S)
CHUNKS = [(0, 4), (4, 4), (8, 4), (12, 4), (16, 2)]
NCHUNK = len(CHUNKS)
EPS = 1e-5
AF = mybir.ActivationFunctionType
OP = mybir.AluOpType


# --------------------------------------------------------------------------
# host-side prep: pure index manipulation / padding / layout
# --------------------------------------------------------------------------

def _host_prep(inputs):
    src = np.asarray(inputs["edge_index"][0]).astype(np.int32)
    dst = np.asarray(inputs["edge_index"][1]).astype(np.int32)
    e2g = np.asarray(inputs["edge2graph"]).astype(np.int32)
    fr = np.asarray(inputs["frac_coords"], np.float32)
    deg = np.bincount(src, minlength=N)
    perm = np.argsort(src, kind="stable")
    srcS, dstS, e2gS = src[perm], dst[perm], e2g[perm]

    # mod-1 wrap bits per edge (exact, from f32 coords)
    d3 = fr[dstS] - fr[srcS]
    kcode = ((d3[:, 0] < 0).astype(np.int32)
             + 2 * (d3[:, 1] < 0).astype(np.int32)
             + 4 * (d3[:, 2] < 0).astype(np.int32))
    cidxS = kcode * G + e2gS

    # edge -> (tile row, slot) fully vectorized: edges sorted by src are
    # contiguous per 128-node tile; slot = edge rank within its tile.
    cum = np.zeros(N + 1, np.int64)
    np.cumsum(deg, out=cum[1:])
    c_of = srcS // NPC
    loc = srcS - c_of * NPC
    nt_of = loc >> 7
    row_of = c_of * NT + nt_of
    first_node = c_of * NPC + (nt_of << 7)
    slot = np.arange(E, dtype=np.int64) - cum[first_node]
    assert slot.max() < ENT, f"node tile overflow: {slot.max() + 1} > {ENT}"
    part = (slot & 127).astype(np.int64)
    col = (slot >> 7).astype(np.int64)

    # dst in low 16 bits, comb index in high 16 (device unpacks via and/shr)
    pidxT = np.zeros((NC * NT, 128, SNT), np.int32)
    pidxT[row_of, part, col] = (cidxS << 16) | dstS

    # per-node degree column (stairs/selT/selp/inv-deg derive on device)
    valid = np.arange(NT * 128) < NPC
    degN = np.zeros((NC, NT * 128), np.float16)
    degN[:, valid] = deg.reshape(NC, NPC).astype(np.float16)
    degN = degN.reshape(NC * NT, 128, 1)

    return dict(pidxT=pidxT, degN=degN)


def _build_frT(fr):
    # frac transposed per own-node tile: frT[c*NT+nt, comp, p] = fr[node, comp]
    valid = np.arange(NT * 128) < NPC
    frT = np.zeros((NC, NT * 128, 4), np.float32)
    frT[:, valid, :3] = fr.reshape(NC, NPC, 3)
    return np.ascontiguousarray(
        frT.reshape(NC * NT, 128, 4).transpose(0, 2, 1)).astype(BF16NP)


def _host_weights(inputs):
    gam = np.asarray(inputs["gamma"], np.float32)
    bet = np.asarray(inputs["beta"], np.float32)
    We1 = np.asarray(inputs["We1"], np.float32)
    Wa, Wb = We1[0:128], We1[128:256]
    Wl, Wf = We1[256:265], We1[265:268]
    be1tot = np.asarray(inputs["be1"], np.float32) + bet @ (Wa + Wb)

    lat = np.asarray(inputs["lattices"], np.float32)
    lat_ip = np.einsum("gij,gkj->gik", lat, lat).reshape(G, 9)
    wlat = lat_ip @ Wl + be1tot  # [G, H]
    kmat = np.array([[(b >> c) & 1 for c in range(3)] for b in range(8)],
                    np.float32)
    kWf = kmat @ Wf  # [8, H]
    comb = (wlat[None, :, :] + kWf[:, None, :]).reshape(8 * G, H)

    def pad4(w):
        out = np.zeros((4, H), np.float32)
        out[:3] = w
        return out

    Wn1 = np.asarray(inputs["Wn1"], np.float32)
    Wn1h, Wn1a = Wn1[0:128], Wn1[128:256]
    bn1tot = np.asarray(inputs["bn1"], np.float32) + bet @ Wn1h

    return dict(
        Wap=(gam[:, None] * Wa).astype(BF16NP),
        Wbp=(gam[:, None] * Wb).astype(BF16NP),
        Wfp=pad4(Wf).astype(BF16NP),
        Wfn=pad4(-Wf).astype(BF16NP),
        comb=comb.astype(BF16NP),
        We2b=np.asarray(inputs["We2"], np.float32).astype(BF16NP),
        be2=np.asarray(inputs["be2"], np.float32),
        Wn1hb=(gam[:, None] * Wn1h).astype(BF16NP),
        Wn1ab=Wn1a.astype(BF16NP),
        bn1t=bn1tot,
        Wn2b=np.asarray(inputs["Wn2"], np.float32).astype(BF16NP),
        bn2=np.asarray(inputs["bn2"], np.float32),
    )


# --------------------------------------------------------------------------
# bass program (single SPMD program for all 8 cores)
# --------------------------------------------------------------------------

def build_program():
    nc = bacc.Bacc()
    p = lambda n, s, d: nc.declare_dram_parameter(n, list(s), d, isOutput=False)

    xown = p("xown", (NPC, H), BF16)
    frT = p("frT", (NT, 4, 128), BF16)
    pidxT = p("pidxT", (NT, 128, SNT), I32)
    degN = p("degN", (NT, 128, 1), FP16)
    comb = p("comb", (8 * G, H), BF16)
    Wap = p("Wap", (H, H), BF16)
    Wbp = p("Wbp", (H, H), BF16)
    Wfp = p("Wfp", (4, H), BF16)
    Wfn = p("Wfn", (4, H), BF16)
    We2b = p("We2b", (H, H), BF16)
    Wn1hb = p("Wn1hb", (H, H), BF16)
    Wn1ab = p("Wn1ab", (H, H), BF16)
    Wn2b = p("Wn2b", (H, H), BF16)
    be2 = p("be2", (H,), F32)
    bn1t = p("bn1t", (H,), F32)
    bn2 = p("bn2", (H,), F32)

    out = nc.declare_dram_parameter("nout", [NPC, H], BF16, isOutput=True)

    with tile.TileContext(nc) as tc:
        with (
            tc.tile_pool(name="dram", bufs=1, space="DRAM") as dram,
            tc.tile_pool(name="persist", bufs=1) as pp,
        ):
            zbslice = dram.tile([NPC, H], BF16)
            zb_tbl = dram.tile([N, H], BF16)

            # ---------------- constants ----------------
            I_bf = pp.tile([128, 128], BF16)
            make_identity(nc, I_bf[:])
            iota_i = pp.tile([128, 512], I32)
            nc.gpsimd.iota(iota_i[:], pattern=[[1, 512]], base=0,
                           channel_multiplier=0)
            iota_f = pp.tile([128, 512], F32)
            nc.any.tensor_copy(out=iota_f[:], in_=iota_i[:])
            # partition-index column and strict-upper-triangular ones matrix
            # (UT[q,p] = 1 iff q < p) for on-device prefix sums of degrees
            iotac_i = pp.tile([128, 1], I32)
            nc.gpsimd.iota(iotac_i[:], pattern=[[1, 1]], base=0,
                           channel_multiplier=1)
            iotac_f = pp.tile([128, 1], F32)
            nc.any.tensor_copy(out=iotac_f[:], in_=iotac_i[:])
            UT_bf = pp.tile([128, 128], BF16)
            nc.vector.tensor_scalar(UT_bf[:], iota_f[:, :128], iotac_f[:],
                                    None, OP.is_gt)

            def load_col(ap, tag):
                t = pp.tile([128, 1], F32, tag=tag)
                nc.sync.dma_start(out=t[:], in_=ap[:, None])
                return t

            be2c = load_col(be2, "be2c")
            bn1c = load_col(bn1t, "bn1c")
            bn2c = load_col(bn2, "bn2c")
            epsc = pp.tile([128, 1], F32)
            nc.gpsimd.memset(epsc[:], EPS)

            def load_w(ap, shape, tag):
                t = pp.tile(list(shape), BF16, tag=tag)
                nc.sync.dma_start(out=t[:], in_=ap[:, :])
                return t

            Wap_s = load_w(Wap, (128, 128), "Wap_s")
            Wbp_s = load_w(Wbp, (128, 128), "Wbp_s")
            Wfp_s = load_w(Wfp, (4, 128), "Wfp_s")
            Wfn_s = load_w(Wfn, (4, 128), "Wfn_s")
            We2_s = load_w(We2b, (128, 128), "We2_s")
            Wn1h_s = load_w(Wn1hb, (128, 128), "Wn1h_s")
            Wn1a_s = load_w(Wn1ab, (128, 128), "Wn1a_s")
            Wn2_s = load_w(Wn2b, (128, 128), "Wn2_s")

            # persistent per-core state
            za_own = pp.tile([128, NT, 128], BF16)
            h0T_own = pp.tile([128, NT, 128], BF16)
            nc.gpsimd.memset(za_own[:], 0.0)
            nc.gpsimd.memset(h0T_own[:], 0.0)

            # ---- phase 1: own nodes -> h0T_own, za_own, zbslice ----
            with (
                tc.tile_pool(name="p1", bufs=3) as pl,
                tc.tile_pool(name="p1psT", bufs=2, space="PSUM") as pps,
                tc.tile_pool(name="p1psZ", bufs=2, space="PSUM") as pps1,
            ):
                for nt in range(NT):
                    rows = 106 if nt == NT - 1 else 128
                    xt_b = pl.tile([128, 128], BF16, tag="xtb")
                    nc.sync.dma_start(out=xt_b[:rows, :],
                                      in_=xown[nt * 128:nt * 128 + rows, :])
                    xt = pl.tile([128, 128], F32, tag="xt")
                    nc.any.tensor_copy(out=xt[:rows, :], in_=xt_b[:rows, :])
                    frt = pl.tile([4, 128], BF16, tag="frt")
                    nc.sync.dma_start(out=frt[:], in_=frT[nt, :, :])
                    st6 = pl.tile([128, 6], F32, tag="st6")
                    nc.vector.bn_stats(st6[:rows, :], xt[:rows, :])
                    st2 = pl.tile([128, 2], F32, tag="st2")
                    nc.vector.bn_aggr(st2[:rows, :], st6[:rows, :])
                    sd = pl.tile([128, 1], F32, tag="sd")
                    nc.scalar.activation(sd[:rows, :], st2[:rows, 1:2],
                                         AF.Sqrt, bias=epsc[:rows, :])
                    a = pl.tile([128, 1], F32, tag="a")
                    nc.vector.reciprocal(a[:rows, :], sd[:rows, :])
                    bnn = pl.tile([128, 1], F32, tag="bnn")
                    nc.vector.tensor_scalar(bnn[:rows, :], st2[:rows, 0:1],
                                            a[:rows, :], -1.0, OP.mult, OP.mult)
                    h0 = pl.tile([128, 128], BF16, tag="h0")
                    nc.scalar.activation(h0[:rows, :], xt[:rows, :],
                                         AF.Identity, bias=bnn[:rows, :],
                                         scale=a[:rows, :])
                    ps_t = pps.tile([128, 128], BF16, tag="psT")
                    nc.tensor.matmul(ps_t[:, :rows], h0[:rows, :],
                                     I_bf[:rows, :rows],
                                     is_transpose=True, start=True, stop=True)
                    nc.any.tensor_copy(out=h0T_own[:, nt, :rows],
                                       in_=ps_t[:, :rows])
                    ps_za = pps1.tile([128, 128], F32, tag="psza")
                    nc.tensor.matmul(ps_za[:], lhsT=h0T_own[:, nt, :],
                                     rhs=Wap_s[:], start=True, stop=False,
                                     skip_group_check=True)
                    nc.tensor.matmul(ps_za[:], lhsT=frt[:], rhs=Wfn_s[:],
                                     start=False, stop=True,
                                     skip_group_check=True)
                    nc.any.tensor_copy(out=za_own[:, nt, :], in_=ps_za[:])
                    ps_zb = pps1.tile([128, 128], F32, tag="pszb")
                    nc.tensor.matmul(ps_zb[:], lhsT=h0T_own[:, nt, :],
                                     rhs=Wbp_s[:], start=True, stop=False,
                                     skip_group_check=True)
                    nc.tensor.matmul(ps_zb[:], lhsT=frt[:], rhs=Wfp_s[:],
                                     start=False, stop=True,
                                     skip_group_check=True)
                    zbb = pl.tile([128, 128], BF16, tag="zbb")
                    nc.any.tensor_copy(out=zbb[:], in_=ps_zb[:])
                    nc.sync.dma_start(out=zbslice[nt * 128:nt * 128 + rows, :],
                                      in_=zbb[:rows, :])

            # ---- share zb across cores ----
            nc.gpsimd.collective_compute(
                "AllGather", OP.bypass,
                replica_groups=[list(range(NC))],
                ins=[zbslice[:].opt()],
                outs=[zb_tbl[:].opt()],
            )

            # ---------------- phase 2: edges + node update ----------------
            with (
                tc.tile_pool(name="idx", bufs=2) as pidx,
                tc.tile_pool(name="gat", bufs=2) as pg,
                tc.tile_pool(name="work", bufs=2) as pw,
                tc.tile_pool(name="ps_z1", bufs=2, space="PSUM") as ps_z1,
                tc.tile_pool(name="ps_z2", bufs=2, space="PSUM") as ps_z2,
                tc.tile_pool(name="ps_agg", bufs=2, space="PSUM") as ps_agg,
                tc.tile_pool(name="ps_sm", bufs=1, space="PSUM") as ps_sm,
            ):
                for nt in range(NT):
                    rows = 106 if nt == NT - 1 else 128
                    # ---- index loads + unpack ----
                    t_pid = pidx.tile([128, SNT], I32, tag="pid")
                    nc.sync.dma_start(out=t_pid[:], in_=pidxT[nt, :, :])
                    t_dst = pidx.tile([128, SNT], I32, tag="dst")
                    nc.vector.tensor_scalar(t_dst[:], t_pid[:], 65535, None,
                                            OP.bitwise_and)
                    t_cid = pidx.tile([128, SNT], I32, tag="cid")
                    nc.vector.tensor_scalar(t_cid[:], t_pid[:], 16, None,
                                            OP.logical_shift_right)

                    # ---- per-node degree -> stair bounds + 1/deg ----
                    dcol16 = pidx.tile([128, 1], FP16, tag="dc16")
                    nc.sync.dma_start(out=dcol16[:], in_=degN[nt, :, :])
                    dcol = pidx.tile([128, 1], F32, tag="dcol")
                    nc.any.tensor_copy(out=dcol[:], in_=dcol16[:])
                    dcol_bf = pidx.tile([128, 1], BF16, tag="dcbf")
                    nc.any.tensor_copy(out=dcol_bf[:], in_=dcol16[:])
                    dmax = pidx.tile([128, 1], F32, tag="dmax")
                    nc.vector.tensor_scalar(dmax[:], dcol[:], 1.0, None,
                                            OP.max)
                    t_invn = pidx.tile([128, 1], F32, tag="invn")
                    nc.vector.reciprocal(t_invn[:], dmax[:])
                    ps_st = ps_sm.tile([128, 1], F32, tag="psst")
                    nc.tensor.matmul(ps_st[:], lhsT=UT_bf[:], rhs=dcol_bf[:],
                                     start=True, stop=True)
                    st_col = pidx.tile([128, 1], F32, tag="stc")
                    nc.any.tensor_copy(out=st_col[:], in_=ps_st[:])
                    en_col = pidx.tile([128, 1], F32, tag="enc")
                    nc.vector.tensor_tensor(out=en_col[:], in0=st_col[:],
                                            in1=dcol[:], op=OP.add)

                    # ---- gathers (edge-major, one row per partition) ----
                    g_zb = pg.tile([128, SNT, 128], BF16, tag="gzb")
                    g_cb = pg.tile([128, SNT, 128], BF16, tag="gcb")
                    for j in range(SNT):
                        nc.gpsimd.indirect_dma_start(
                            out=g_zb[:, j, :], out_offset=None,
                            in_=zb_tbl[:, :],
                            in_offset=bass.IndirectOffsetOnAxis(
                                ap=t_dst[:, j:j + 1], axis=0))
                        nc.gpsimd.indirect_dma_start(
                            out=g_cb[:, j, :], out_offset=None,
                            in_=comb[:, :],
                            in_offset=bass.IndirectOffsetOnAxis(
                                ap=t_cid[:, j:j + 1], axis=0))

                    agg = ps_agg.tile([128, 128], F32, tag="agg")

                    for ci, (j0, S) in enumerate(CHUNKS):
                        W = S * 128
                        base = float(j0 * 128)
                        stb = pw.tile([128, 1], F32, tag="stb")
                        nc.vector.tensor_scalar(stb[:], st_col[:], base, None,
                                                OP.subtract)
                        enb = pw.tile([128, 1], F32, tag="enb")
                        nc.vector.tensor_scalar(enb[:], en_col[:], base, None,
                                                OP.subtract)
                        # staircase selection matrix selT [128n, W]
                        t0 = pw.tile([128, 512], BF16, tag="t0")
                        nc.vector.tensor_scalar(
                            t0[:, :W], iota_f[:, :W], enb[:], None, OP.is_lt)
                        selT = pw.tile([128, 512], BF16, tag="selT")
                        nc.vector.scalar_tensor_tensor(
                            out=selT[:, :W], in0=iota_f[:, :W],
                            scalar=stb[:], in1=t0[:, :W],
                            op0=OP.is_ge, op1=OP.mult)
                        # selT with 1/deg folded per node row (scatter-mean)
                        selTs = pw.tile([128, 512], BF16, tag="selTs")
                        nc.scalar.activation(selTs[:, :W], selT[:, :W],
                                             AF.Identity, scale=t_invn[:])

                        # zb + comb summed, then xbar-transposed to FM
                        gsum = pw.tile([128, 4, 128], BF16, tag="gsum")
                        nc.vector.tensor_tensor(
                            out=gsum[:, :S, :], in0=g_zb[:, j0:j0 + S, :],
                            in1=g_cb[:, j0:j0 + S, :], op=OP.add)
                        gT = pw.tile([128, 4, 128], BF16, tag="gT")
                        nc.sync.dma_start_transpose(gT[:, :S, :],
                                                    gsum[:, :S, :])

                        # z1T accumulation [128H, W]
                        z1 = ps_z1.tile([128, 512], F32, tag="z1")
                        nc.tensor.matmul(z1[:, :W], lhsT=za_own[:, nt, :],
                                         rhs=selT[:, :W], start=True,
                                         stop=False, skip_group_check=True)
                        nc.tensor.matmul(z1[:, :W], lhsT=I_bf[:],
                                         rhs=gT[:, :S, :], start=False,
                                         stop=True, skip_group_check=True)

                        e1T = pw.tile([128, 512], BF16, tag="e1T")
                        nc.scalar.activation(e1T[:, :W], z1[:, :W], AF.Silu)

                        z2 = ps_z2.tile([128, 512], F32, tag="z2")
                        nc.tensor.matmul(z2[:, :W], lhsT=We2_s[:],
                                         rhs=e1T[:, :W], start=True, stop=True)
                        e2T = pw.tile([128, 512], BF16, tag="e2T")
                        nc.scalar.activation(e2T[:, :W], z2[:, :W], AF.Silu,
                                             bias=be2c[:])
                        e2em = pw.tile([128, 4, 128], BF16, tag="e2em")
                        nc.sync.dma_start_transpose(e2em[:, :S, :], e2T[:, :W])

                        # scatter-mean matmuls into agg [128H, 128n]:
                        # selp = (selTs subchunk)^T via PE transpose
                        for j in range(S):
                            ps_sp = ps_sm.tile([128, 128], BF16, tag="pssp")
                            nc.tensor.matmul(
                                ps_sp[:], selTs[:, j * 128:(j + 1) * 128],
                                I_bf[:], is_transpose=True,
                                start=True, stop=True)
                            selp = pw.tile([128, 128], BF16, tag="selp")
                            nc.any.tensor_copy(out=selp[:], in_=ps_sp[:])
                            nc.tensor.matmul(
                                agg[:], lhsT=e2em[:, j, :], rhs=selp[:],
                                start=(ci == 0 and j == 0),
                                stop=(ci == NCHUNK - 1 and j == S - 1),
                                skip_group_check=True)

                    # ---- node update for this tile ----
                    aggb = pw.tile([128, 128], BF16, tag="aggb")
                    nc.any.tensor_copy(out=aggb[:], in_=agg[:])
                    n1 = ps_z1.tile([128, 512], F32, tag="z1")
                    nc.tensor.matmul(n1[:, :128], lhsT=Wn1h_s[:],
                                     rhs=h0T_own[:, nt, :], start=True,
                                     stop=False, skip_group_check=True)
                    nc.tensor.matmul(n1[:, :128], lhsT=Wn1a_s[:], rhs=aggb[:],
                                     start=False, stop=True,
                                     skip_group_check=True)
                    n1T = pw.tile([128, 128], BF16, tag="n1T")
                    nc.scalar.activation(n1T[:], n1[:, :128], AF.Silu,
                                         bias=bn1c[:])
                    n2 = ps_z2.tile([128, 512], F32, tag="z2")
                    nc.tensor.matmul(n2[:, :128], lhsT=Wn2_s[:], rhs=n1T[:],
                                     start=True, stop=True)
                    n2T = pw.tile([128, 128], BF16, tag="n2T")
                    nc.scalar.activation(n2T[:], n2[:, :128], AF.Silu,
                                         bias=bn2c[:])
                    n2em = pw.tile([128, 1, 128], BF16, tag="n2em")
                    nc.sync.dma_start_transpose(n2em[:], n2T[:])
                    nc.sync.dma_start(out=out[nt * 128:nt * 128 + rows, :],
                                      in_=n2em[:rows, 0, :])
    nc.finalize()
    return nc


# --------------------------------------------------------------------------
# cached jit runner (trace/lower/compile once per process)
# --------------------------------------------------------------------------

class _Result:
    exec_time_ns = None
    profile_json = None
    mean_exec_time_ns = None
    results = None


class _Runner:
    def __init__(self):
        import jax
        import jax.numpy as jnp
        from jax.sharding import Mesh, PartitionSpec, NamedSharding
        from jax.experimental.shard_map import shard_map
        from concourse.bass2jax import (
            _bass_exec_p, install_neuronx_cc_hook, partition_id_tensor)

        self.jax = jax
        nc = build_program()
        self.nc = nc
        install_neuronx_cc_hook()

        partition_name = (nc.partition_id_tensor.name
                          if nc.partition_id_tensor else None)
        in_names, out_names, out_avals = [], [], []
        for alloc in nc.m.functions[0].allocations:
            if not isinstance(alloc, mybir.MemoryLocationSet):
                continue
            name = alloc.memorylocations[0].name
            if alloc.kind == "ExternalInput":
                if name != partition_name:
                    in_names.append(name)
            elif alloc.kind == "ExternalOutput":
                out_names.append(name)
                out_avals.append(jax.core.ShapedArray(
                    tuple(alloc.tensor_shape), mybir.dt.np(alloc.dtype)))
        self.in_names, self.out_names = in_names, out_names
        n_params, n_outs = len(in_names), len(out_avals)
        all_in = tuple(in_names + out_names
                       + ([partition_name] if partition_name else []))

        def _body(*args):
            operands = list(args)
            if partition_name is not None:
                operands.append(partition_id_tensor())
            outs = _bass_exec_p.bind(
                *operands, out_avals=tuple(out_avals), in_names=all_in,
                out_names=tuple(out_names), lowering_input_output_aliases=(),
                sim_require_finite=True, sim_require_nnan=True, nc=nc)
            return tuple(outs)

        devices = jax.devices()[:NC]
        assert len(devices) == NC
        mesh = Mesh(np.asarray(devices), ("core",))
        PS = PartitionSpec
        donate = tuple(range(n_params, n_params + n_outs))
        self.fn = jax.jit(
            shard_map(_body, mesh=mesh,
                      in_specs=(PS("core"),) * (n_params + n_outs),
                      out_specs=(PS("core"),) * n_outs, check_rep=False),
            donate_argnums=donate, keep_unused=True)

        sh = NamedSharding(mesh, PS("core"))
        self.sh = sh
        zshapes = [(NC * a.shape[0], *a.shape[1:]) for a in out_avals]
        zdtypes = [a.dtype for a in out_avals]
        self.make_zeros = jax.jit(
            lambda: tuple(jnp.zeros(s, d) for s, d in zip(zshapes, zdtypes)),
            out_shardings=(sh,) * n_outs)
        self._zeros = None

    def __call__(self, arg_map):
        args = [arg_map[n] for n in self.in_names]
        zeros = self._zeros if self._zeros is not None else self.make_zeros()
        self._zeros = None
        outs = self.fn(*args, *zeros)
        # pre-make donated zero buffers for the next call (async on device)
        self._zeros = self.make_zeros()
        return {name: outs[i] for i, name in enumerate(self.out_names)}

    @staticmethod
    def fetch(arr):
        # per-shard parallel device->host pull: ~70x faster than a
        # sequential np.asarray on the global array under axon
        from concurrent.futures import ThreadPoolExecutor
        shards = arr.addressable_shards
        with ThreadPoolExecutor(len(shards)) as ex:
            parts = list(ex.map(lambda s: np.asarray(s.data), shards))
        return np.concatenate(parts, axis=0)


_RUNNER = None


def kernel(**inputs) -> np.ndarray:
    out, _ = run(inputs, trace=False)
    return out


def run(inputs, trace=False):
    import threading
    import jax

    global _RUNNER
    if _RUNNER is None:
        _RUNNER = _Runner()
    R = _RUNNER

    x = np.ascontiguousarray(np.asarray(inputs["node_features"], np.float32))
    fr = np.asarray(inputs["frac_coords"], np.float32)

    # args that need no edge prep: upload eagerly in a worker thread while
    # the main thread does the edge indexing (device_put is lazy unless
    # blocked on, hence the explicit block inside the thread)
    early = dict(xown=x.astype(BF16NP), frT=_build_frT(fr))
    for k, v in _host_weights(inputs).items():
        early[k] = np.tile(v, (NC,) + (1,) * (v.ndim - 1)) if v.ndim > 1 \
            else np.tile(v, NC)
    dev = {}

    def put_early():
        for k, v in early.items():
            dev[k] = jax.device_put(v, R.sh)
        jax.block_until_ready(list(dev.values()))

    th = threading.Thread(target=put_early)
    th.start()
    idx = _host_prep(inputs)
    th.join()

    am = dict(dev)
    am.update(idx)
    outs = R(am)
    nout = _Runner.fetch(outs["nout"])  # [N, H] bf16, core-order == node-order
    res = _Result()
    return x + nout.astype(np.float32), res


if __name__ == "__main__":
    build_program()
    print("program built OK")
